# revision 51
# baseline (speedup 1.0000x reference)
"""Trainium2 Bass kernel for nn_DWTEnhancedSTGCN (B=8, T=12, N=10000, E=160000).

Strategy (N-sharded over 8 NeuronCores), I/O-minimized:
  - The axon tunnel re-streams every input (and the pre-zeroed output
    buffers) on each execute, and on-device compute is ~free, so the design
    minimizes per-call bytes: fp16 node features / index payloads / outputs,
    weights packed to one 13-row copy (the per-batch [128,128] blocks of the
    old layout all held identical content), structural constants (identity
    blocks, half-selectors, ones) generated on device with iota/memset, the
    pre-zeroed output operands dropped (every output element is written),
    and the full-graph gather table built ON DEVICE: each core ships only
    its own x slice, PE-transposes it locally, and an AllGather of the
    transposed stripes assembles the [node, feature] fp16 table in HBM.
    Outputs are written f32 (D2H fetch is untimed; dropping the f16
    down-convert relieves the ACT engine). Gathers spread over 4 SWDGE
    queues; dense-phase pools are deep enough to pipeline across groups;
    stats staging loads are Pool-issued to offload the SP sequencer, and
    the LN stats matmul weights are pre-divided by OD so means come
    straight off the PE. Stats tiles use 4-row pair groups so the sumsq
    psum rows land in two contiguous DMAs.
  - Each core owns 1250 dst-nodes for ALL 8 batch elements; each edge's 96
    batch-features (+ones) are gathered once per core via dma_gather (256B
    fp16 rows from the on-device table).
  - Aggregation = mean over in-edges, folded into one-hot segment-sum
    matmuls on the PE: onehot[e, j] = (iota[j] == dstloc[e]) * invdeg
    (one fused DVE tensor_scalar), then aggT_block += G_chunk.T @ onehot.
  - Dense phase in [feature, node] layout; per-batch 13-row rhs tiles
    (12 x-features + ones/indicator row) contract against the packed
    weights. LayerNorm stats via ones-matmul column sums, batched across
    node-chunks; activations on ACT; fusion on DVE.
Host does only: sharding/reshapes, integer index-stream building, and
parameter-only weight folding. All FP math on x runs on device.
"""
import sys
import numpy as np

sys.path.insert(0, '/opt/trn_rl_repo')

B, T, N = 8, 12, 10000
OD = 64
NCORE = 8
NPC = N // NCORE          # 1250 nodes per core
NPCP = 1280               # padded local node count (10 blocks of 128)
NBLK = NPCP // 128
EPS = 1e-5
PADROW = NPCP - 2         # all-zero pad row (stripe-0 tail) for padding idxs
NROWS = NCORE * NPCP      # table rows: per-core 1280-row stripes
CHUNKS = [(0, 512), (512, 512), (1024, 256)]
NPAIR = B * len(CHUNKS)   # 24
GRP = 8                   # pairs per stats/softmax group
NGRP = NPAIR // GRP
REPEAT = 1                # in-kernel repetition (timing mode)
NQ_ALLOC = 4              # SWDGE queues allocated (1..4)
NQ_SPREAD = 4             # queues the gathers round-robin over (<= NQ_ALLOC)
XTS_SHARED = True         # AllGather output in Shared addr space

# packed-weight column layout: [16, PWC] fp16; rows 0:12 = weight rows,
# row 12 = bias row (pairs with the ones/indicator row of the rhs tiles)
PW = {'w1': slice(0, 128), 'w2': slice(128, 256), 'w3': slice(256, 320),
      'wr': slice(320, 384), 'wsx': slice(384, 387), 'wsab': slice(387, 389),
      'wsac': slice(389, 391)}
PWC = 391

_cache = {}


# ----------------------------------------------------------------- host prep
def _prep_graph(edge_index):
    src = np.asarray(edge_index[0]).astype(np.int64).ravel()
    dst = np.asarray(edge_index[1]).astype(np.int64).ravel()
    deg = np.bincount(dst, minlength=N)
    invdeg = (1.0 / np.maximum(deg, 1)).astype(np.float32)
    order = np.argsort(dst, kind='stable')
    s_s, d_s = src[order], dst[order]
    core = d_s // NPC
    local = d_s - core * NPC
    blk = local >> 7
    dstloc = local & 127
    binid = core * NBLK + blk
    counts = np.bincount(binid, minlength=NCORE * NBLK)
    return dict(s=s_s, d=d_s, core=core, binid=binid, dstloc=dstloc,
                blk=blk, counts=counts, invdeg=invdeg)


def _build_streams(g, b_pad, nseg, nch):
    stream = NBLK * b_pad
    starts = np.zeros(NCORE * NBLK, np.int64)
    np.cumsum(g['counts'][:-1], out=starts[1:])
    rank = np.arange(len(g['s'])) - starts[g['binid']]
    pos = g['core'] * stream + g['blk'] * b_pad + rank
    src_stream = np.full(NCORE * stream, PADROW, np.int64)
    dl_stream = np.zeros(NCORE * stream, np.float32)
    iv_stream = np.zeros(NCORE * stream, np.float32)
    # table rows are per-core 1280-row stripes: row = core*NPCP + local
    src_stream[pos] = (g['s'] // NPC) * NPCP + (g['s'] % NPC)
    dl_stream[pos] = g['dstloc']
    iv_stream[pos] = g['invdeg'][g['d']]
    idxs, dlis = [], []
    for c in range(NCORE):
        st = src_stream[c * stream:(c + 1) * stream]
        stp = np.full(nch * 1024, PADROW, np.int64)
        stp[:stream] = st
        t16 = stp.reshape(nch, 64, 16).transpose(2, 0, 1).reshape(16, nch * 64)
        idxs.append(np.ascontiguousarray(t16.astype(np.int16)))
        dl = dl_stream[c * stream:(c + 1) * stream].reshape(nseg, 128).T
        iv = iv_stream[c * stream:(c + 1) * stream].reshape(nseg, 128).T
        dlis.append(np.ascontiguousarray(
            np.concatenate([dl, iv], axis=1).astype(np.float16)))
    return idxs, dlis


def _pack_weights(p):
    f = lambda k: np.asarray(p[k], np.float32)
    h16 = lambda a: a.astype(np.float16)
    W_ht, b_ht, W_lt, b_lt = f('W_ht'), f('b_ht'), f('W_lt'), f('b_lt')
    Ws_h, Wn_h, b_h = f('Ws_h'), f('Wn_h'), f('b_h')
    Ws_l, Wn_l, Wc_l, b_l = f('Ws_l'), f('Wn_l'), f('Wc_l'), f('b_l')
    Whr, bhr, Wlr, blr = f('Whr'), f('bhr'), f('Wlr'), f('blr')
    Wg, bg = f('Wg'), f('bg')
    W1 = h16(np.concatenate([W_ht @ (Ws_h + 0.2 * Whr),
                             W_lt @ (Ws_l + 0.2 * Wlr)], 1))
    W1b = h16(np.concatenate([b_ht @ (Ws_h + 0.2 * Whr) + b_h + 0.2 * bhr,
                              b_lt @ (Ws_l + 0.2 * Wlr) + b_l + 0.2 * blr]))
    W2 = h16(np.concatenate([W_ht @ Wn_h, W_lt @ Wn_l], 1))
    W2b = h16(np.concatenate([b_ht @ Wn_h, b_lt @ Wn_l]))
    W3 = h16(W_lt @ Wc_l)
    W3b = h16(b_lt @ Wc_l)
    WR = h16(2.0 * Wg)
    WRb = h16(bg)
    # column sums of the f16-rounded matrices, so the on-device mean matches
    # the f16 matmul results up to one extra rounding
    s32 = lambda a: a.astype(np.float32)
    pw = np.zeros((16, PWC), np.float16)
    pw[0:12, PW['w1']], pw[12, PW['w1']] = W1, W1b
    pw[0:12, PW['w2']], pw[12, PW['w2']] = W2, W2b
    pw[0:12, PW['w3']], pw[12, PW['w3']] = W3, W3b
    pw[0:12, PW['wr']], pw[12, PW['wr']] = WR, WRb
    # stats weights pre-divided by OD so the PE emits means directly
    r = 1.0 / OD
    pw[0:12, PW['wsx']] = h16(r * np.stack(
        [s32(W1[:, 0:64]).sum(1), s32(W1[:, 64:128]).sum(1),
         s32(WR).sum(1)], 1))
    pw[12, PW['wsx']] = h16(r * np.array(
        [s32(W1b[0:64]).sum(), s32(W1b[64:128]).sum(), s32(WRb).sum()]))
    pw[0:12, PW['wsab']] = h16(r * np.stack(
        [s32(W2[:, 0:64]).sum(1), s32(W2[:, 64:128]).sum(1)], 1))
    pw[12, PW['wsab']] = h16(r * np.array(
        [s32(W2b[0:64]).sum(), s32(W2b[64:128]).sum()]))
    pw[0:12, PW['wsac']] = h16(r * np.stack(
        [np.zeros(T, np.float32), s32(W3).sum(1)], 1))
    pw[12, PW['wsac']] = h16(r * np.array([0.0, s32(W3b).sum()]))

    pv = np.zeros((128, 8), np.float32)
    pv[:, 0] = np.concatenate([f('g_hn'), f('g_ln')])
    pv[:, 1] = np.concatenate([f('b_hn'), f('b_ln')])
    pv[:, 2] = f('Wa')[:, 0] - f('Wa')[:, 1]
    pv[0:64, 3] = 0.1 * f('g_gn')
    pv[0:64, 4] = 0.1 * f('b_gn')
    ba = f('ba')
    return pw, pv, float(ba[0] - ba[1])


# -------------------------------------------------------------- bass program
def _build_program(nseg, nch, ba_diff, repeat=1):
    import concourse.tile as tile
    from concourse import bacc, mybir

    f32 = mybir.dt.float32
    f16 = mybir.dt.float16
    i16 = mybir.dt.int16
    AF = mybir.ActivationFunctionType
    OP = mybir.AluOpType
    SEG_PER_BLK = nseg // NBLK

    nc = bacc.Bacc("TRN2", target_bir_lowering=False, debug=False,
                   enable_asserts=False, num_devices=NCORE,
                   num_swdge_queues=NQ_ALLOC)

    # single packed input buffer per core (fewer PJRT buffers = less
    # execute-path jitter); sections are f16-viewed flat byte ranges
    off = {}
    _o = 0
    for name, n in [('xsh', 96 * NPCP), ('pw', 16 * PWC), ('pv', 128 * 16),
                    ('idx_b', 16 * nch * 64), ('idx_c', 16 * nch * 64),
                    ('dli_b', 128 * 2 * nseg), ('dli_c', 128 * 2 * nseg)]:
        off[name] = (_o, n)
        _o += n
    BLOB = _o
    blob_d = nc.dram_tensor("blob", [1, BLOB], f16, kind="ExternalInput")

    def sect(name, dt, cols):
        o, n = off[name]
        ap = blob_d.ap()[0:1, o:o + n]
        if dt is not f16:
            ap = ap.bitcast(dt)
        return ap.rearrange("a (r c) -> (a r) c", c=cols)

    xsh_ap = sect('xsh', f16, NPCP)
    pw_ap = sect('pw', f16, PWC)
    pv_ap = sect('pv', f32, 8)
    idx_ap = {g: sect(f'idx_{g}', i16, nch * 64) for g in "bc"}
    dli_ap = {g: sect(f'dli_{g}', f16, 2 * nseg) for g in "bc"}
    # gather table built on device: PE-transpose the OWN x slice into
    # [node, feature] rows, then AllGather the per-core transposed stripes
    # straight into the final table (per-core 1280-row stripes; pad rows
    # are zero because the x slice's pad columns are host-zeroed).
    xtsl_d = nc.dram_tensor("xtsl", [NPCP, 128], f16, kind="Internal")
    xts_d = nc.dram_tensor("xts", [NROWS, 128], f16, kind="Internal",
                           addr_space="Shared" if XTS_SHARED else "Local")
    out_d = nc.dram_tensor("out", [3, B, OD, NPC], f32, kind="ExternalOutput")

    def mmg(mms):
        """Emit matmuls as one PSUM accumulation group.
        mms: list of (out_ap, lhsT_ap, rhs_ap, tile_position)."""
        nmm = len(mms)
        for i, (out, lhsT, rhs, tp) in enumerate(mms):
            nc.tensor.matmul(out, lhsT, rhs, start=(i == 0),
                             stop=(i == nmm - 1), skip_group_check=True,
                             tile_position=tp)

    with tile.TileContext(nc) as tc:
        with (
            tc.tile_pool(name="const", bufs=1) as cpool,
        ):
            pw_t = cpool.tile([16, PWC], f16, tag="pw")
            nc.sync.dma_start(pw_t[:], pw_ap)
            pv_t = cpool.tile([128, 8], f32, tag="pv")
            nc.sync.dma_start(pv_t[:], pv_ap)

            w1 = pw_t[0:13, PW['w1']]
            w2 = pw_t[0:13, PW['w2']]
            w3 = pw_t[0:13, PW['w3']]
            wr = pw_t[0:13, PW['wr']]
            wsx = pw_t[0:13, PW['wsx']]
            wsab = pw_t[0:13, PW['wsab']]
            wsac = pw_t[0:13, PW['wsac']]

            # structural constants, generated on device (single-partition
            # rows built at partition 0, then DMA'd into place — compute
            # engines cannot start at unaligned partitions)
            ec1 = cpool.tile([1, 128], f32, tag="ec1")
            nc.gpsimd.memset(ec1[:, 0:64], 0.0)
            nc.gpsimd.memset(ec1[:, 64:128], 1.0)
            # copy of ec1 at partition 32: pairs with the w1 row of the
            # merged softmax tile (PE needs equal lhsT/rhs base partitions)
            ec132 = cpool.tile([33, 128], f32, tag="ec132")
            nc.gpsimd.memset(ec132[32:33, 0:64], 0.0)
            nc.gpsimd.memset(ec132[32:33, 64:128], 1.0)
            ehl = cpool.tile([2, 128], f32, tag="ehl")
            nc.sync.dma_start(ehl[1:2, :], ec1[:])
            onesr_t = cpool.tile([1, 64], f32, tag="onesr")
            nc.gpsimd.memset(onesr_t[:], 1.0)
            # sumsq reducers pre-scaled by 1/OD (stats arrive as means);
            # ones64 col 1 is zero so the r-sumsq occupies rows 64:66 of the
            # stats psum and the {32,33,64,65} rows form one regular pattern
            ones64_t = cpool.tile([64, 2], f32, tag="ones64")
            nc.gpsimd.memset(ones64_t[:], 0.0)
            nc.gpsimd.memset(ones64_t[:, 0:1], 1.0 / OD)
            oneshl_t = cpool.tile([128, 2], f32, tag="oneshl")
            nc.gpsimd.memset(oneshl_t[:], 0.0)
            nc.gpsimd.memset(oneshl_t[0:64, 0:1], 1.0 / OD)
            nc.gpsimd.memset(oneshl_t[64:128, 1:2], 1.0 / OD)
            ident2 = cpool.tile([128, 64], f32, tag="ident2")
            eps_t = cpool.tile([32, 1], f32, tag="eps")
            nc.gpsimd.memset(eps_t[:], EPS)
            bad_t = cpool.tile([GRP, 1], f32, tag="bad")
            nc.gpsimd.memset(bad_t[:], ba_diff)
            c13_t = cpool.tile([GRP, 1], f32, tag="c13")
            nc.gpsimd.memset(c13_t[:], 1.3)

            # per-batch 13-row x tiles: rows 0:12 features, row 12 ones
            xb_t = [cpool.tile([16, NPCP], f16, tag=f"xb{b}", name=f"xb{b}")
                    for b in range(B)]
            aggT = {g: cpool.tile([128, NPCP], f16, tag=f"agg{g}",
                                  name=f"aggT{g}") for g in "bc"}
            for g in "bc":
                nc.gpsimd.memset(aggT[g][:], 0.0)

            # ---- gather + one-hot segment-sum (per graph) ----
            agb_t = {'b': [], 'c': []}
            for _rep in range(repeat):
              with (
                  tc.tile_pool(name="ld", bufs=1) as ldpool,
                  tc.tile_pool(name="gat", bufs=4) as gpool,
                  tc.tile_pool(name="oh", bufs=8) as ohpool,
                  tc.tile_pool(name="aggps", bufs=2, space="PSUM") as aggps,
              ):
                # gather-scoped loads + on-device constant builds (the pool
                # frees before the dense-phase pools open)
                xsh_t = ldpool.tile([96, NPCP], f16, tag="xsh")
                nc.sync.dma_start(xsh_t[:], xsh_ap)
                ones_h = ldpool.tile([1, NPCP], f16, tag="onesh")
                nc.gpsimd.memset(ones_h[:], 1.0)
                ec0 = ldpool.tile([1, 128], f32, tag="ec0")
                nc.gpsimd.memset(ec0[:, 0:64], 1.0)
                nc.gpsimd.memset(ec0[:, 64:128], 0.0)
                nc.sync.dma_start(ehl[0:1, :], ec0[:])
                for b in range(B):
                    nc.sync.dma_start(xb_t[b][0:12, :],
                                      xsh_t[12 * b:12 * b + 12, :])
                    nc.sync.dma_start(xb_t[b][12:13, :], ones_h[:])
                iota_h = ldpool.tile([128, 128], f16, tag="iotah")
                nc.gpsimd.iota(iota_h[:], pattern=[[1, 128]], base=0,
                               channel_multiplier=0,
                               allow_small_or_imprecise_dtypes=True)
                ic_t = ldpool.tile([128, 64], f32, tag="ic")
                nc.gpsimd.iota(ic_t[:], pattern=[[1, 64]], base=0,
                               channel_multiplier=0,
                               allow_small_or_imprecise_dtypes=True)
                ip_t = ldpool.tile([128, 1], f32, tag="ip")
                nc.gpsimd.iota(ip_t[:], pattern=[[1, 1]], base=0,
                               channel_multiplier=1,
                               allow_small_or_imprecise_dtypes=True)
                ige_t = ldpool.tile([128, 1], f32, tag="ige")
                nc.vector.tensor_scalar(ige_t[:], ip_t[:], 64.0, None,
                                        OP.is_ge)
                ipm_t = ldpool.tile([128, 1], f32, tag="ipm")
                nc.vector.scalar_tensor_tensor(ipm_t[:], ige_t[:], -64.0,
                                               ip_t[:], OP.mult, OP.add)
                nc.vector.tensor_scalar(ident2[:], ic_t[:], ipm_t[:], None,
                                        OP.is_equal)
                idx_t, dli_t = {}, {}
                for g in "bc":
                    idx_t[g] = ldpool.tile([128, nch * 64], i16,
                                           tag=f"idx{g}", name=f"idx{g}")
                    for c in range(8):
                        nc.sync.dma_start(idx_t[g][16 * c:16 * c + 16, :],
                                          idx_ap[g])
                    dli16 = ldpool.tile([128, 2 * nseg], f16,
                                        tag=f"dli16{g}", name=f"dli16{g}")
                    nc.sync.dma_start(dli16[:], dli_ap[g])
                    dli_t[g] = ldpool.tile([128, 2 * nseg], f32,
                                           tag=f"dli{g}", name=f"dli{g}")
                    nc.scalar.activation(dli_t[g][:], dli16[:], AF.Copy)

                # ---- build the gather table on device ----
                # transpose the OWN slab locally, then AllGather the
                # transposed [1280, 128] stripes straight into the table
                ide16 = ldpool.tile([128, 128], f16, tag="ide16")
                nc.vector.tensor_scalar(ide16[:], iota_h[:], ip_t[:], None,
                                        OP.is_equal)
                slab = ldpool.tile([128, NPCP], f16, tag="slab")
                nc.gpsimd.memset(slab[:], 0.0)
                nc.sync.dma_start(slab[0:96, :], xsh_t[:])
                nc.sync.dma_start(slab[96:97, :], ones_h[:])
                with (
                    tc.tile_pool(name="tb", bufs=4) as tbpool,
                    tc.tile_pool(name="tps", bufs=4, space="PSUM") as tpps,
                ):
                    for k in range(NBLK):
                        pst = tpps.tile([128, 128], f16, tag="pst")
                        nc.tensor.transpose(
                            pst[:], slab[:, 128 * k:128 * k + 128],
                            ide16[:])
                        tsb = tbpool.tile([128, 128], f16, tag="tsb")
                        nc.scalar.activation(tsb[:], pst[:], AF.Copy)
                        nc.sync.dma_start(
                            xtsl_d.ap()[128 * k:128 * k + 128, :], tsb[:])
                tc.strict_bb_all_engine_barrier()
                nc.gpsimd.collective_compute(
                    "AllGather", mybir.AluOpType.bypass,
                    replica_groups=[list(range(NCORE))],
                    ins=[xtsl_d.ap()], outs=[xts_d.ap()])
                tc.strict_bb_all_engine_barrier()

                for g in "bc":
                  ps_blk = None
                  for k in range(nch):
                      gt = gpool.tile([128, 8 * 128], f16, tag="g")
                      gt3 = gt[:].rearrange("p (c e) -> p c e", e=128)
                      nc.gpsimd.dma_gather(
                          gt3, xts_d.ap(),
                          idx_t[g][:, k * 64:(k + 1) * 64],
                          num_idxs=1024, num_idxs_reg=1024, elem_size=128,
                          queue_num=k % NQ_SPREAD)
                      for c in range(8):
                          s = k * 8 + c
                          if s >= nseg:
                              break
                          r = s % SEG_PER_BLK
                          j = s // SEG_PER_BLK
                          if r == 0:
                              ps_blk = aggps.tile([128, 128], f32, tag="agg")
                          oh = ohpool.tile([128, 128], f16, tag="oh")
                          nc.vector.tensor_scalar(
                              oh[:], iota_h[:],
                              dli_t[g][:, s:s + 1],
                              dli_t[g][:, nseg + s:nseg + s + 1],
                              OP.is_equal, OP.mult)
                          nc.tensor.matmul(
                              ps_blk[0:97, :], gt3[:, c, 0:97], oh[:],
                              start=(r == 0), stop=(r == SEG_PER_BLK - 1),
                              skip_group_check=True, tile_position=(0, 0))
                          if r == SEG_PER_BLK - 1:
                              nc.scalar.activation(
                                  aggT[g][0:97, j * 128:(j + 1) * 128],
                                  ps_blk[0:97, :], AF.Copy)

              # per-batch 13-row agg tiles: rows 0:12 agg features, row 12
              # the deg>0 indicator (invdeg-weighted ones-row aggregate)
              for g in "bc":
                  agb_t[g] = []
                  for b in range(B):
                      ag = cpool.tile([16, NPCP], f16, tag=f"ag{g}{b}",
                                      name=f"ag{g}{b}")
                      nc.sync.dma_start(ag[0:12, :],
                                        aggT[g][12 * b:12 * b + 12, :])
                      nc.sync.dma_start(ag[12:13, :], aggT[g][96:97, :])
                      agb_t[g].append(ag)

              # ---- dense phase in groups of GRP pairs ----
              pairs = [(b, c0, kl) for b in range(B) for (c0, kl) in CHUNKS]
              with (
                  tc.tile_pool(name="mainps", bufs=2, space="PSUM") as mainps,
                  tc.tile_pool(name="statps", bufs=2, space="PSUM") as statps,
                  tc.tile_pool(name="ebc", bufs=2, space="PSUM") as ebcps,
                  tc.tile_pool(name="shl", bufs=GRP + 1) as shlpool,
                  tc.tile_pool(name="sr", bufs=GRP + 1) as srpool,
                  tc.tile_pool(name="hla", bufs=GRP + 1) as hlapool,
                  tc.tile_pool(name="sq", bufs=2) as sqpool,
                  tc.tile_pool(name="ssb", bufs=2) as ssbpool,
                  tc.tile_pool(name="stg", bufs=2) as stgpool,
                  tc.tile_pool(name="tmp", bufs=2) as tmppool,
                  tc.tile_pool(name="stat", bufs=2) as statpool,
                  tc.tile_pool(name="smax", bufs=2) as smaxpool,
              ):
               for grp in range(NGRP):
                  gpairs = list(enumerate(pairs[grp * GRP:(grp + 1) * GRP]))
                  # stats tiles in 4-row pair groups so the sumsq psum rows
                  # {32,33,64,65} land with ONE regular-pattern DMA per pair;
                  # st1/st2/aux are separate base-0 tiles (TensorTensor needs
                  # equal input base partitions)
                  st1 = statpool.tile([32, 512], f32, tag="st1")
                  st2 = statpool.tile([32, 512], f32, tag="st2")
                  sdt = smaxpool.tile([GRP, 512], f32, tag="sdt")
                  nc.gpsimd.memset(st1[:], 0.0)
                  nc.gpsimd.memset(st2[:], 1.0)
                  nc.gpsimd.memset(sdt[:], 0.0)
                  shl_t, sr_t, hla_t = {}, {}, {}

                  for q, (b, c0, kl) in gpairs:
                      xr = xb_t[b][0:13, c0:c0 + kl]
                      ab = agb_t['b'][b][0:13, c0:c0 + kl]
                      ac = agb_t['c'][b][0:13, c0:c0 + kl]

                      phl = mainps.tile([128, 512], f32, tag="phl")
                      mmg([(phl[:, 0:kl], w1, xr, (0, 0)),
                           (phl[:, 0:kl], w2, ab, (0, 0)),
                           (phl[64:128, 0:kl], w3, ac, (0, 64))])
                      pres = mainps.tile([64, 512], f32, tag="pres")
                      mmg([(pres[:, 0:kl], wr, xr, (0, 0))])

                      sh = shlpool.tile([128, 512], f16, tag="shl")
                      shl_t[q] = sh
                      nc.scalar.activation(sh[:, 0:kl], phl[:, 0:kl], AF.Copy)
                      sr = srpool.tile([64, 512], f16, tag="sr",
                                       name=f"sr{q}")
                      sr_t[q] = sr
                      nc.scalar.activation(sr[0:64, 0:kl],
                                           pres[:, 0:kl], AF.Copy)
                      sq = sqpool.tile([128, 512], f32, tag="sq")
                      nc.scalar.activation(sq[:, 0:kl], sh[:, 0:kl], AF.Square)
                      sqr = sqpool.tile([64, 512], f32, tag="sqr")
                      nc.scalar.activation(sqr[:, 0:kl],
                                           sr[0:64, 0:kl], AF.Square)

                      # stats psum (already /OD): means@0:3, meansq_hl@32:34,
                      # meansq_r@64:66 (row 65 is a zero column of ones64)
                      S = statps.tile([66, 512], f32, tag="S")
                      mmg([(S[0:3, 0:kl], wsx, xr, (0, 0)),
                           (S[0:2, 0:kl], wsab, ab, (0, 0)),
                           (S[0:2, 0:kl], wsac, ac, (0, 0))])
                      mmg([(S[32:34, 0:kl], oneshl_t[:], sq[:, 0:kl],
                            (0, 32))])
                      mmg([(S[64:66, 0:kl], ones64_t[:], sqr[:, 0:kl],
                            (0, 64))])
                      ssb = ssbpool.tile([96, 512], f32, tag="ssb")
                      nc.scalar.activation(ssb[0:66, 0:kl], S[:, 0:kl],
                                           AF.Copy)
                      nc.sync.dma_start(st1[4 * q:4 * q + 3, 0:kl],
                                        ssb[0:3, 0:kl])
                      nc.sync.dma_start(st2[4 * q:4 * q + 2, 0:kl],
                                        ssb[32:34, 0:kl])
                      nc.sync.dma_start(st2[4 * q + 2:4 * q + 4, 0:kl],
                                        ssb[64:66, 0:kl])

                  # ---- batched stats math (in-place to save SBUF) ----
                  # st1 = means; st2 meansq -> var -> rstd; aux m^2 -> std
                  # -> m*rstd
                  aux = statpool.tile([32, 512], f32, tag="aux")
                  nc.vector.tensor_mul(aux[:], st1[:], st1[:])
                  nc.vector.tensor_sub(st2[:], st2[:], aux[:])
                  nc.scalar.activation(aux[:], st2[:], AF.Sqrt,
                                       bias=eps_t[:])
                  nc.vector.reciprocal(st2[:], aux[:])
                  nc.vector.tensor_mul(aux[:], st1[:], st2[:])
                  rstd, mrstd = st2, aux

                  # ---- per-pair LN apply + activations + logit diff ----
                  for q, (b, c0, kl) in gpairs:
                      sh = shl_t[q]
                      # Pool-issued staging loads offload the SP sequencer
                      rstg = stgpool.tile([2, 512], f32, tag="rstg")
                      nc.gpsimd.dma_start(rstg[:, 0:kl],
                                          rstd[4 * q:4 * q + 2, 0:kl])
                      mstg = stgpool.tile([2, 512], f32, tag="mstg")
                      nc.gpsimd.dma_start(mstg[:, 0:kl],
                                          mrstd[4 * q:4 * q + 2, 0:kl])
                      rbc = ebcps.tile([128, 512], f32, tag="ebc")
                      nc.tensor.matmul(rbc[:, 0:kl], ehl[:],
                                       rstg[:, 0:kl],
                                       start=True, stop=True,
                                       skip_group_check=True,
                                       tile_position=(0, 0))
                      mbc = ebcps.tile([128, 512], f32, tag="ebc")
                      nc.tensor.matmul(mbc[:, 0:kl], ehl[:],
                                       mstg[:, 0:kl],
                                       start=True, stop=True,
                                       skip_group_check=True,
                                       tile_position=(0, 0))
                      t1 = tmppool.tile([128, 512], f32, tag="t1")
                      nc.vector.tensor_mul(t1[:, 0:kl], sh[:, 0:kl],
                                           rbc[:, 0:kl])
                      t2 = tmppool.tile([128, 512], f32, tag="t2")
                      nc.vector.tensor_sub(t2[:, 0:kl], t1[:, 0:kl],
                                           mbc[:, 0:kl])
                      hla = hlapool.tile([128, 512], f32, tag="hla")
                      hla_t[q] = hla
                      yh = tmppool.tile([64, 512], f32, tag="yh")
                      nc.scalar.activation(yh[:, 0:kl], t2[0:64, 0:kl],
                                           AF.Identity,
                                           bias=pv_t[0:64, 1:2],
                                           scale=pv_t[0:64, 0:1])
                      nc.vector.scalar_tensor_tensor(
                          hla[0:64, 0:kl], yh[:, 0:kl], 0.1, yh[:, 0:kl],
                          OP.mult, OP.max)
                      nc.scalar.activation(hla[64:128, 0:kl], t2[64:128, 0:kl],
                                           AF.Gelu,
                                           bias=pv_t[64:128, 1:2],
                                           scale=pv_t[64:128, 0:1])
                      klo = min(kl, NPC - c0)
                      nc.sync.dma_start(out_d.ap()[1, b, :, c0:c0 + klo],
                                        hla[0:64, 0:klo])
                      nc.sync.dma_start(out_d.ap()[2, b, :, c0:c0 + klo],
                                        hla[64:128, 0:klo])
                      sd = statps.tile([1, 512], f32, tag="S")
                      nc.tensor.matmul(sd[:, 0:kl], pv_t[:, 2:3],
                                       hla[:, 0:kl],
                                       start=True, stop=True,
                                       skip_group_check=True,
                                       tile_position=(0, 0))
                      sdb = ssbpool.tile([1, 512], f32, tag="sdb")
                      nc.scalar.activation(sdb[:, 0:kl], sd[:, 0:kl], AF.Copy)
                      nc.sync.dma_start(sdt[q:q + 1, 0:kl], sdb[:, 0:kl])

                  # ---- batched 2-way softmax (in-place to save SBUF) ----
                  a0 = smaxpool.tile([GRP, 512], f32, tag="a0")
                  nc.scalar.activation(a0[:], sdt[:], AF.Sigmoid,
                                       bias=bad_t[:])
                  w0 = sdt
                  nc.vector.tensor_scalar_add(w0[:], a0[:], 0.3)
                  w1_ = a0
                  nc.scalar.activation(w1_[:], a0[:], AF.Identity,
                                       bias=c13_t[:], scale=-1.0)

                  # ---- per-pair fusion + residual + output ----
                  for q, (b, c0, kl) in gpairs:
                      hla = hla_t[q]
                      sr = sr_t[q]
                      w0s = stgpool.tile([1, 512], f32, tag="w0s")
                      nc.sync.dma_start(w0s[:, 0:kl], w0[q:q + 1, 0:kl])
                      w1s = stgpool.tile([1, 512], f32, tag="w1s")
                      nc.sync.dma_start(w1s[:, 0:kl], w1_[q:q + 1, 0:kl])
                      wbc = ebcps.tile([128, 512], f32, tag="ebc")
                      nc.tensor.matmul(wbc[:, 0:kl], ehl[0:1, :],
                                       w0s[:, 0:kl], start=True,
                                       stop=False, skip_group_check=True,
                                       tile_position=(0, 0))
                      nc.tensor.matmul(wbc[:, 0:kl], ec1[:],
                                       w1s[:, 0:kl], start=False,
                                       stop=True, skip_group_check=True,
                                       tile_position=(0, 0))
                      f1 = tmppool.tile([128, 512], f32, tag="f1")
                      nc.vector.tensor_mul(f1[:, 0:kl], hla[:, 0:kl],
                                           wbc[:, 0:kl])
                      rrs = stgpool.tile([1, 512], f32, tag="rrs")
                      nc.gpsimd.dma_start(rrs[:, 0:kl],
                                          rstd[4 * q + 2:4 * q + 3, 0:kl])
                      rms = stgpool.tile([1, 512], f32, tag="rms")
                      nc.gpsimd.dma_start(rms[:, 0:kl],
                                          mrstd[4 * q + 2:4 * q + 3, 0:kl])
                      rr = ebcps.tile([64, 512], f32, tag="ebc")
                      nc.tensor.matmul(rr[:, 0:kl], onesr_t[:],
                                       rrs[:, 0:kl],
                                       start=True, stop=True,
                                       skip_group_check=True,
                                       tile_position=(0, 0))
                      rm = ebcps.tile([64, 512], f32, tag="ebc")
                      nc.tensor.matmul(rm[:, 0:kl], onesr_t[:],
                                       rms[:, 0:kl],
                                       start=True, stop=True,
                                       skip_group_check=True,
                                       tile_position=(0, 0))
                      u1 = tmppool.tile([64, 512], f32, tag="u1")
                      nc.vector.tensor_mul(u1[:, 0:kl], sr[0:64, 0:kl],
                                           rr[:, 0:kl])
                      u2 = tmppool.tile([64, 512], f32, tag="u2")
                      nc.vector.tensor_sub(u2[:, 0:kl], u1[:, 0:kl],
                                           rm[:, 0:kl])
                      resa = tmppool.tile([64, 512], f32, tag="resa")
                      nc.scalar.activation(resa[:, 0:kl], u2[:, 0:kl],
                                           AF.Identity,
                                           bias=pv_t[0:64, 4:5],
                                           scale=pv_t[0:64, 3:4])
                      f2 = ebcps.tile([64, 512], f32, tag="ebc")
                      nc.tensor.matmul(f2[:, 0:kl], ident2[:], f1[:, 0:kl],
                                       start=True, stop=True,
                                       skip_group_check=True,
                                       tile_position=(0, 0))
                      f3 = tmppool.tile([64, 512], f32, tag="f3")
                      nc.vector.tensor_add(f3[:, 0:kl], f2[:, 0:kl],
                                           resa[:, 0:kl])
                      klo = min(kl, NPC - c0)
                      nc.sync.dma_start(out_d.ap()[0, b, :, c0:c0 + klo],
                                        f3[:, 0:klo])
    nc.finalize()
    return nc


# ------------------------------------------------------------------- runner
class _SpmdRunner:
    def __init__(self, nc, n_cores=NCORE):
        import jax
        from jax.sharding import Mesh, PartitionSpec
        from jax.experimental.shard_map import shard_map
        from concourse import mybir
        from concourse.bass2jax import (_bass_exec_p, install_neuronx_cc_hook,
                                        partition_id_tensor)
        install_neuronx_cc_hook()
        self.jax = jax
        self.n_cores = n_cores
        partition_name = (nc.partition_id_tensor.name
                          if nc.partition_id_tensor else None)
        in_names, out_names, out_avals = [], [], []
        for alloc in nc.m.functions[0].allocations:
            if not isinstance(alloc, mybir.MemoryLocationSet):
                continue
            name = alloc.memorylocations[0].name
            if alloc.kind == "ExternalInput":
                if name != partition_name:
                    in_names.append(name)
            elif alloc.kind == "ExternalOutput":
                out_names.append(name)
                shape = tuple(alloc.tensor_shape)
                dtype = mybir.dt.np(alloc.dtype)
                out_avals.append(jax.core.ShapedArray(shape, dtype))
        self.in_names, self.out_names = in_names, out_names
        self.out_avals = out_avals
        n_params = len(in_names)
        # The kernel writes every element of every output, so the pre-zeroed
        # output operands of the stock runner are dropped — they would be
        # re-streamed to the terminal on every execute.
        all_in = list(in_names)
        if partition_name is not None:
            all_in.append(partition_name)

        def _body(*args):
            operands = list(args)
            if partition_name is not None:
                operands.append(partition_id_tensor())
            outs = _bass_exec_p.bind(
                *operands, out_avals=tuple(out_avals),
                in_names=tuple(all_in), out_names=tuple(out_names),
                lowering_input_output_aliases=(),
                sim_require_finite=True, sim_require_nnan=True, nc=nc)
            return tuple(outs)

        devices = jax.devices()[:n_cores]
        mesh = Mesh(np.asarray(devices), ("core",))
        in_specs = (PartitionSpec("core"),) * n_params
        out_specs = (PartitionSpec("core"),) * len(out_names)
        self.fn = jax.jit(
            shard_map(_body, mesh=mesh, in_specs=in_specs,
                      out_specs=out_specs, check_rep=False),
            keep_unused=True)

    def prepare(self, in_maps):
        n = self.n_cores
        per_core = [[np.ascontiguousarray(m[name]) for name in self.in_names]
                    for m in in_maps]
        concat_in = [np.concatenate([per_core[c][i] for c in range(n)], axis=0)
                     for i in range(len(self.in_names))]
        return [self.jax.device_put(a) for a in concat_in]

    def run(self, args):
        outs = self.fn(*args)
        self.jax.block_until_ready(outs)
        return outs

    def split_outs(self, outs):
        res = []
        for c in range(self.n_cores):
            d = {}
            for i, name in enumerate(self.out_names):
                d[name] = np.asarray(outs[i]).reshape(
                    self.n_cores, *self.out_avals[i].shape)[c]
            res.append(d)
        return res


# -------------------------------------------------------------------- entry
def _get(inputs):
    gb = _prep_graph(inputs['edge_index'])
    gc = _prep_graph(inputs['causal_edge_index'])
    b_pad = max(128, -(-int(max(gb['counts'].max(), gc['counts'].max()))
                     // 128) * 128)
    stream = NBLK * b_pad
    nseg = stream // 128
    nch = -(-stream // 1024)
    pw, pv, ba_diff = _pack_weights(inputs)
    key = (b_pad, nseg, nch, round(ba_diff, 9), REPEAT,
           NQ_ALLOC, NQ_SPREAD, XTS_SHARED)
    if key not in _cache:
        nc = _build_program(nseg, nch, ba_diff, REPEAT)
        _cache[key] = _SpmdRunner(nc)
    return _cache[key], gb, gc, b_pad, nseg, nch, pw, pv


def make_in_maps(inputs):
    runner, gb, gc, b_pad, nseg, nch, pw, pv = _get(inputs)
    x = np.asarray(inputs['x'], np.float32)
    xflat = x.reshape(96, N)
    idx_b, dli_b = _build_streams(gb, b_pad, nseg, nch)
    idx_c, dli_c = _build_streams(gc, b_pad, nseg, nch)
    h = lambda a: np.ascontiguousarray(a).view(np.float16).ravel()
    in_maps = []
    for c in range(NCORE):
        xsh = np.zeros((96, NPCP), np.float16)
        xsh[:, 0:NPC] = xflat[:, c * NPC:(c + 1) * NPC]
        blob = np.concatenate([
            h(xsh), h(pw), h(pv),
            h(idx_b[c]), h(idx_c[c]),
            h(dli_b[c]), h(dli_c[c]),
        ])[None, :]
        in_maps.append({'blob': blob})
    return runner, in_maps


def kernel(**inputs):
    runner, in_maps = make_in_maps(inputs)
    args = runner.prepare(in_maps)
    outs = runner.run(args)
    res = runner.split_outs(outs)
    fused = np.empty((B, OD, N), np.float32)
    high = np.empty((B, OD, N), np.float32)
    low = np.empty((B, OD, N), np.float32)
    for c in range(NCORE):
        sl = slice(c * NPC, (c + 1) * NPC)
        o = res[c]['out'].astype(np.float32)
        fused[:, :, sl] = o[0]
        high[:, :, sl] = o[1]
        low[:, :, sl] = o[2]
    return fused, high, low



# revision 55
# speedup vs baseline: 1.1025x; 1.1025x over previous
"""Trainium2 Bass kernel for nn_DWTEnhancedSTGCN (B=8, T=12, N=10000, E=160000).

Strategy (N-sharded over 8 NeuronCores), I/O-minimized:
  - The axon tunnel re-streams every input (and the pre-zeroed output
    buffers) on each execute, and on-device compute is ~free, so the design
    minimizes per-call bytes: fp16 node features / index payloads / outputs,
    weights packed to one 13-row copy (the per-batch [128,128] blocks of the
    old layout all held identical content), structural constants (identity
    blocks, half-selectors, ones) generated on device with iota/memset, the
    pre-zeroed output operands dropped (every output element is written),
    and the full-graph gather table built ON DEVICE: each core ships only
    its own x slice, PE-transposes it locally, and an AllGather of the
    transposed stripes assembles the [node, feature] fp16 table in HBM.
    Outputs are written f32 (D2H fetch is untimed; dropping the f16
    down-convert relieves the ACT engine). Gathers spread over 4 SWDGE
    queues; dense-phase pools are deep enough to pipeline across groups;
    stats staging loads are Pool-issued to offload the SP sequencer, and
    the LN stats matmul weights are pre-divided by OD so means come
    straight off the PE. Stats tiles use 4-row pair groups so the sumsq
    psum rows land in two contiguous DMAs.
  - Each core owns 1250 dst-nodes for ALL 8 batch elements; each edge's 96
    batch-features (+ones) are gathered once per core via dma_gather (256B
    fp16 rows from the on-device table).
  - Aggregation = mean over in-edges, folded into one-hot segment-sum
    matmuls on the PE: onehot[e, j] = (iota[j] == dstloc[e]) * invdeg
    (one fused DVE tensor_scalar), then aggT_block += G_chunk.T @ onehot.
  - Dense phase in [feature, node] layout; per-batch 13-row rhs tiles
    (12 x-features + ones/indicator row) contract against the packed
    weights. LayerNorm stats via ones-matmul column sums, batched across
    node-chunks; activations on ACT; fusion on DVE.
Host does only: sharding/reshapes, integer index-stream building, and
parameter-only weight folding. All FP math on x runs on device.
"""
import sys
import numpy as np

sys.path.insert(0, '/opt/trn_rl_repo')

B, T, N = 8, 12, 10000
OD = 64
NCORE = 8
NPC = N // NCORE          # 1250 nodes per core
NPCP = 1280               # padded local node count (10 blocks of 128)
NBLK = NPCP // 128
EPS = 1e-5
PADROW = NPCP - 2         # all-zero pad row (stripe-0 tail) for padding idxs
NROWS = NCORE * NPCP      # table rows: per-core 1280-row stripes
CHUNKS = [(0, 512), (512, 512), (1024, 256)]
NPAIR = B * len(CHUNKS)   # 24
GRP = 8                   # pairs per stats/softmax group
NGRP = NPAIR // GRP
REPEAT = 1                # in-kernel repetition (timing mode)
NQ_ALLOC = 4              # SWDGE queues allocated (1..4)
NQ_SPREAD = 4             # queues the gathers round-robin over (<= NQ_ALLOC)
XTS_SHARED = True         # AllGather output in Shared addr space

# packed-weight column layout: [16, PWC] fp16; rows 0:12 = weight rows,
# row 12 = bias row (pairs with the ones/indicator row of the rhs tiles)
PW = {'w1': slice(0, 128), 'w2': slice(128, 256), 'w3': slice(256, 320),
      'wr': slice(320, 384), 'wsx': slice(384, 387), 'wsab': slice(387, 389),
      'wsac': slice(389, 391)}
PWC = 391

_cache = {}


# ----------------------------------------------------------------- host prep
def _prep_graph(edge_index):
    src = np.asarray(edge_index[0]).astype(np.int64).ravel()
    dst = np.asarray(edge_index[1]).astype(np.int64).ravel()
    deg = np.bincount(dst, minlength=N)
    invdeg = (1.0 / np.maximum(deg, 1)).astype(np.float32)
    order = np.argsort(dst, kind='stable')
    s_s, d_s = src[order], dst[order]
    core = d_s // NPC
    local = d_s - core * NPC
    blk = local >> 7
    dstloc = local & 127
    binid = core * NBLK + blk
    counts = np.bincount(binid, minlength=NCORE * NBLK)
    return dict(s=s_s, d=d_s, core=core, binid=binid, dstloc=dstloc,
                blk=blk, counts=counts, invdeg=invdeg)


def _build_streams(g, b_pad, nseg, nch):
    stream = NBLK * b_pad
    starts = np.zeros(NCORE * NBLK, np.int64)
    np.cumsum(g['counts'][:-1], out=starts[1:])
    rank = np.arange(len(g['s'])) - starts[g['binid']]
    pos = g['core'] * stream + g['blk'] * b_pad + rank
    src_stream = np.full(NCORE * stream, PADROW, np.int64)
    dl_stream = np.zeros(NCORE * stream, np.float32)
    iv_stream = np.zeros(NCORE * stream, np.float32)
    # table rows are per-core 1280-row stripes: row = core*NPCP + local
    src_stream[pos] = (g['s'] // NPC) * NPCP + (g['s'] % NPC)
    dl_stream[pos] = g['dstloc']
    iv_stream[pos] = g['invdeg'][g['d']]
    idxs, dlis = [], []
    for c in range(NCORE):
        st = src_stream[c * stream:(c + 1) * stream]
        stp = np.full(nch * 1024, PADROW, np.int64)
        stp[:stream] = st
        t16 = stp.reshape(nch, 64, 16).transpose(2, 0, 1).reshape(16, nch * 64)
        idxs.append(np.ascontiguousarray(t16.astype(np.int16)))
        dl = dl_stream[c * stream:(c + 1) * stream].reshape(nseg, 128).T
        iv = iv_stream[c * stream:(c + 1) * stream].reshape(nseg, 128).T
        dlis.append(np.ascontiguousarray(
            np.concatenate([dl, iv], axis=1).astype(np.float16)))
    return idxs, dlis


def _pack_weights(p):
    f = lambda k: np.asarray(p[k], np.float32)
    h16 = lambda a: a.astype(np.float16)
    W_ht, b_ht, W_lt, b_lt = f('W_ht'), f('b_ht'), f('W_lt'), f('b_lt')
    Ws_h, Wn_h, b_h = f('Ws_h'), f('Wn_h'), f('b_h')
    Ws_l, Wn_l, Wc_l, b_l = f('Ws_l'), f('Wn_l'), f('Wc_l'), f('b_l')
    Whr, bhr, Wlr, blr = f('Whr'), f('bhr'), f('Wlr'), f('blr')
    Wg, bg = f('Wg'), f('bg')
    W1 = h16(np.concatenate([W_ht @ (Ws_h + 0.2 * Whr),
                             W_lt @ (Ws_l + 0.2 * Wlr)], 1))
    W1b = h16(np.concatenate([b_ht @ (Ws_h + 0.2 * Whr) + b_h + 0.2 * bhr,
                              b_lt @ (Ws_l + 0.2 * Wlr) + b_l + 0.2 * blr]))
    W2 = h16(np.concatenate([W_ht @ Wn_h, W_lt @ Wn_l], 1))
    W2b = h16(np.concatenate([b_ht @ Wn_h, b_lt @ Wn_l]))
    W3 = h16(W_lt @ Wc_l)
    W3b = h16(b_lt @ Wc_l)
    WR = h16(2.0 * Wg)
    WRb = h16(bg)
    # column sums of the f16-rounded matrices, so the on-device mean matches
    # the f16 matmul results up to one extra rounding
    s32 = lambda a: a.astype(np.float32)
    pw = np.zeros((16, PWC), np.float16)
    pw[0:12, PW['w1']], pw[12, PW['w1']] = W1, W1b
    pw[0:12, PW['w2']], pw[12, PW['w2']] = W2, W2b
    pw[0:12, PW['w3']], pw[12, PW['w3']] = W3, W3b
    pw[0:12, PW['wr']], pw[12, PW['wr']] = WR, WRb
    # stats weights pre-divided by OD so the PE emits means directly
    r = 1.0 / OD
    pw[0:12, PW['wsx']] = h16(r * np.stack(
        [s32(W1[:, 0:64]).sum(1), s32(W1[:, 64:128]).sum(1),
         s32(WR).sum(1)], 1))
    pw[12, PW['wsx']] = h16(r * np.array(
        [s32(W1b[0:64]).sum(), s32(W1b[64:128]).sum(), s32(WRb).sum()]))
    pw[0:12, PW['wsab']] = h16(r * np.stack(
        [s32(W2[:, 0:64]).sum(1), s32(W2[:, 64:128]).sum(1)], 1))
    pw[12, PW['wsab']] = h16(r * np.array(
        [s32(W2b[0:64]).sum(), s32(W2b[64:128]).sum()]))
    pw[0:12, PW['wsac']] = h16(r * np.stack(
        [np.zeros(T, np.float32), s32(W3).sum(1)], 1))
    pw[12, PW['wsac']] = h16(r * np.array([0.0, s32(W3b).sum()]))

    pv = np.zeros((128, 8), np.float32)
    pv[:, 0] = np.concatenate([f('g_hn'), f('g_ln')])
    pv[:, 1] = np.concatenate([f('b_hn'), f('b_ln')])
    pv[:, 2] = f('Wa')[:, 0] - f('Wa')[:, 1]
    pv[0:64, 3] = 0.1 * f('g_gn')
    pv[0:64, 4] = 0.1 * f('b_gn')
    ba = f('ba')
    return pw, pv, float(ba[0] - ba[1])


# -------------------------------------------------------------- bass program
def _build_program(nseg, nch, ba_diff, repeat=1):
    import concourse.tile as tile
    from concourse import bacc, mybir

    f32 = mybir.dt.float32
    f16 = mybir.dt.float16
    i16 = mybir.dt.int16
    AF = mybir.ActivationFunctionType
    OP = mybir.AluOpType
    SEG_PER_BLK = nseg // NBLK

    nc = bacc.Bacc("TRN2", target_bir_lowering=False, debug=False,
                   enable_asserts=False, num_devices=NCORE,
                   num_swdge_queues=NQ_ALLOC)

    # single packed input buffer per core (fewer PJRT buffers = less
    # execute-path jitter); sections are f16-viewed flat byte ranges
    off = {}
    _o = 0
    for name, n in [('xsh', 96 * NPCP), ('pw', 16 * PWC), ('pv', 128 * 16),
                    ('idx_b', 16 * nch * 64), ('idx_c', 16 * nch * 64),
                    ('dli_b', 128 * 2 * nseg), ('dli_c', 128 * 2 * nseg)]:
        off[name] = (_o, n)
        _o += n
    BLOB = _o
    blob_d = nc.dram_tensor("blob", [1, BLOB], f16, kind="ExternalInput")

    def sect(name, dt, cols):
        o, n = off[name]
        ap = blob_d.ap()[0:1, o:o + n]
        if dt is not f16:
            ap = ap.bitcast(dt)
        return ap.rearrange("a (r c) -> (a r) c", c=cols)

    xsh_ap = sect('xsh', f16, NPCP)
    pw_ap = sect('pw', f16, PWC)
    pv_ap = sect('pv', f32, 8)
    idx_ap = {g: sect(f'idx_{g}', i16, nch * 64) for g in "bc"}
    dli_ap = {g: sect(f'dli_{g}', f16, 2 * nseg) for g in "bc"}
    # gather table built on device: PE-transpose the OWN x slice into
    # [node, feature] rows, then AllGather the per-core transposed stripes
    # straight into the final table (per-core 1280-row stripes; pad rows
    # are zero because the x slice's pad columns are host-zeroed).
    xtsl_d = nc.dram_tensor("xtsl", [NPCP, 128], f16, kind="Internal")
    xts_d = nc.dram_tensor("xts", [NROWS, 128], f16, kind="Internal",
                           addr_space="Shared" if XTS_SHARED else "Local")
    out_d = nc.dram_tensor("out", [3, B, OD, NPC], f32, kind="ExternalOutput")

    def mmg(mms):
        """Emit matmuls as one PSUM accumulation group.
        mms: list of (out_ap, lhsT_ap, rhs_ap, tile_position)."""
        nmm = len(mms)
        for i, (out, lhsT, rhs, tp) in enumerate(mms):
            nc.tensor.matmul(out, lhsT, rhs, start=(i == 0),
                             stop=(i == nmm - 1), skip_group_check=True,
                             tile_position=tp)

    with tile.TileContext(nc) as tc:
        with (
            tc.tile_pool(name="const", bufs=1) as cpool,
        ):
            pw_t = cpool.tile([16, PWC], f16, tag="pw")
            nc.sync.dma_start(pw_t[:], pw_ap)
            pv_t = cpool.tile([128, 8], f32, tag="pv")
            nc.sync.dma_start(pv_t[:], pv_ap)

            w1 = pw_t[0:13, PW['w1']]
            w2 = pw_t[0:13, PW['w2']]
            w3 = pw_t[0:13, PW['w3']]
            wr = pw_t[0:13, PW['wr']]
            wsx = pw_t[0:13, PW['wsx']]
            wsab = pw_t[0:13, PW['wsab']]
            wsac = pw_t[0:13, PW['wsac']]

            # structural constants, generated on device (single-partition
            # rows built at partition 0, then DMA'd into place — compute
            # engines cannot start at unaligned partitions)
            ec1 = cpool.tile([1, 128], f32, tag="ec1")
            nc.gpsimd.memset(ec1[:, 0:64], 0.0)
            nc.gpsimd.memset(ec1[:, 64:128], 1.0)
            # copy of ec1 at partition 32: pairs with the w1 row of the
            # merged softmax tile (PE needs equal lhsT/rhs base partitions)
            ec132 = cpool.tile([33, 128], f32, tag="ec132")
            nc.gpsimd.memset(ec132[32:33, 0:64], 0.0)
            nc.gpsimd.memset(ec132[32:33, 64:128], 1.0)
            ehl = cpool.tile([2, 128], f32, tag="ehl")
            nc.sync.dma_start(ehl[1:2, :], ec1[:])
            onesr_t = cpool.tile([1, 64], f32, tag="onesr")
            nc.gpsimd.memset(onesr_t[:], 1.0)
            # sumsq reducers pre-scaled by 1/OD (stats arrive as means);
            # ones64 col 1 is zero so the r-sumsq occupies rows 64:66 of the
            # stats psum and the {32,33,64,65} rows form one regular pattern
            ones64_t = cpool.tile([64, 2], f32, tag="ones64")
            nc.gpsimd.memset(ones64_t[:], 0.0)
            nc.gpsimd.memset(ones64_t[:, 0:1], 1.0 / OD)
            oneshl_t = cpool.tile([128, 2], f32, tag="oneshl")
            nc.gpsimd.memset(oneshl_t[:], 0.0)
            nc.gpsimd.memset(oneshl_t[0:64, 0:1], 1.0 / OD)
            nc.gpsimd.memset(oneshl_t[64:128, 1:2], 1.0 / OD)
            ident2 = cpool.tile([128, 64], f32, tag="ident2")
            eps_t = cpool.tile([32, 1], f32, tag="eps")
            nc.gpsimd.memset(eps_t[:], EPS)
            bad_t = cpool.tile([GRP, 1], f32, tag="bad")
            nc.gpsimd.memset(bad_t[:], ba_diff)
            c13_t = cpool.tile([GRP, 1], f32, tag="c13")
            nc.gpsimd.memset(c13_t[:], 1.3)

            # per-batch 13-row x tiles: rows 0:12 features, row 12 ones
            xb_t = [cpool.tile([16, NPCP], f16, tag=f"xb{b}", name=f"xb{b}")
                    for b in range(B)]
            aggT = {g: cpool.tile([128, NPCP], f16, tag=f"agg{g}",
                                  name=f"aggT{g}") for g in "bc"}
            for g in "bc":
                nc.gpsimd.memset(aggT[g][:], 0.0)

            # ---- gather + one-hot segment-sum (per graph) ----
            agb_t = {'b': [], 'c': []}
            for _rep in range(repeat):
              with (
                  tc.tile_pool(name="ld", bufs=1) as ldpool,
                  tc.tile_pool(name="gat", bufs=4) as gpool,
                  tc.tile_pool(name="oh", bufs=8) as ohpool,
                  tc.tile_pool(name="aggps", bufs=4, space="PSUM") as aggps,
              ):
                # gather-scoped loads + on-device constant builds (the pool
                # frees before the dense-phase pools open)
                xsh_t = ldpool.tile([96, NPCP], f16, tag="xsh")
                nc.sync.dma_start(xsh_t[:], xsh_ap)
                ones_h = ldpool.tile([1, NPCP], f16, tag="onesh")
                nc.gpsimd.memset(ones_h[:], 1.0)
                ec0 = ldpool.tile([1, 128], f32, tag="ec0")
                nc.gpsimd.memset(ec0[:, 0:64], 1.0)
                nc.gpsimd.memset(ec0[:, 64:128], 0.0)
                nc.sync.dma_start(ehl[0:1, :], ec0[:])
                for b in range(B):
                    nc.sync.dma_start(xb_t[b][0:12, :],
                                      xsh_t[12 * b:12 * b + 12, :])
                    nc.sync.dma_start(xb_t[b][12:13, :], ones_h[:])
                iota_h = ldpool.tile([128, 128], f16, tag="iotah")
                nc.gpsimd.iota(iota_h[:], pattern=[[1, 128]], base=0,
                               channel_multiplier=0,
                               allow_small_or_imprecise_dtypes=True)
                ic_t = ldpool.tile([128, 64], f32, tag="ic")
                nc.gpsimd.iota(ic_t[:], pattern=[[1, 64]], base=0,
                               channel_multiplier=0,
                               allow_small_or_imprecise_dtypes=True)
                ip_t = ldpool.tile([128, 1], f32, tag="ip")
                nc.gpsimd.iota(ip_t[:], pattern=[[1, 1]], base=0,
                               channel_multiplier=1,
                               allow_small_or_imprecise_dtypes=True)
                ige_t = ldpool.tile([128, 1], f32, tag="ige")
                nc.vector.tensor_scalar(ige_t[:], ip_t[:], 64.0, None,
                                        OP.is_ge)
                ipm_t = ldpool.tile([128, 1], f32, tag="ipm")
                nc.vector.scalar_tensor_tensor(ipm_t[:], ige_t[:], -64.0,
                                               ip_t[:], OP.mult, OP.add)
                nc.vector.tensor_scalar(ident2[:], ic_t[:], ipm_t[:], None,
                                        OP.is_equal)
                idx_t, dli_t = {}, {}
                for g in "bc":
                    idx_t[g] = ldpool.tile([128, nch * 64], i16,
                                           tag=f"idx{g}", name=f"idx{g}")
                    for c in range(8):
                        nc.sync.dma_start(idx_t[g][16 * c:16 * c + 16, :],
                                          idx_ap[g])
                    dli16 = ldpool.tile([128, 2 * nseg], f16,
                                        tag=f"dli16{g}", name=f"dli16{g}")
                    nc.sync.dma_start(dli16[:], dli_ap[g])
                    dli_t[g] = ldpool.tile([128, 2 * nseg], f32,
                                           tag=f"dli{g}", name=f"dli{g}")
                    nc.scalar.activation(dli_t[g][:], dli16[:], AF.Copy)

                # ---- build the gather table on device ----
                # transpose the OWN slab locally, then AllGather the
                # transposed [1280, 128] stripes straight into the table
                ide16 = ldpool.tile([128, 128], f16, tag="ide16")
                nc.vector.tensor_scalar(ide16[:], iota_h[:], ip_t[:], None,
                                        OP.is_equal)
                slab = ldpool.tile([128, NPCP], f16, tag="slab")
                nc.gpsimd.memset(slab[:], 0.0)
                nc.sync.dma_start(slab[0:96, :], xsh_t[:])
                nc.sync.dma_start(slab[96:97, :], ones_h[:])
                with (
                    tc.tile_pool(name="tb", bufs=4) as tbpool,
                    tc.tile_pool(name="tps", bufs=4, space="PSUM") as tpps,
                ):
                    for k in range(NBLK):
                        pst = tpps.tile([128, 128], f16, tag="pst")
                        nc.tensor.transpose(
                            pst[:], slab[:, 128 * k:128 * k + 128],
                            ide16[:])
                        tsb = tbpool.tile([128, 128], f16, tag="tsb")
                        nc.scalar.activation(tsb[:], pst[:], AF.Copy)
                        nc.sync.dma_start(
                            xtsl_d.ap()[128 * k:128 * k + 128, :], tsb[:])
                tc.strict_bb_all_engine_barrier()
                nc.gpsimd.collective_compute(
                    "AllGather", mybir.AluOpType.bypass,
                    replica_groups=[list(range(NCORE))],
                    ins=[xtsl_d.ap()], outs=[xts_d.ap()])
                tc.strict_bb_all_engine_barrier()

                # graphs interleaved chunk-by-chunk so BOTH graphs' dst
                # blocks complete progressively and the dense phase can
                # start on early columns while late chunks still gather
                ps_blk = {}
                for k in range(nch):
                  for gi, g in enumerate("bc"):
                      gt = gpool.tile([128, 8 * 128], f16, tag="g")
                      gt3 = gt[:].rearrange("p (c e) -> p c e", e=128)
                      nc.gpsimd.dma_gather(
                          gt3, xts_d.ap(),
                          idx_t[g][:, k * 64:(k + 1) * 64],
                          num_idxs=1024, num_idxs_reg=1024, elem_size=128,
                          queue_num=(2 * k + gi) % NQ_SPREAD)
                      for c in range(8):
                          s = k * 8 + c
                          if s >= nseg:
                              break
                          r = s % SEG_PER_BLK
                          j = s // SEG_PER_BLK
                          if r == 0:
                              ps_blk[g] = aggps.tile([128, 128], f32,
                                                     tag="agg",
                                                     name=f"agg{g}{j}")
                          oh = ohpool.tile([128, 128], f16, tag="oh")
                          nc.vector.tensor_scalar(
                              oh[:], iota_h[:],
                              dli_t[g][:, s:s + 1],
                              dli_t[g][:, nseg + s:nseg + s + 1],
                              OP.is_equal, OP.mult)
                          nc.tensor.matmul(
                              ps_blk[g][0:97, :], gt3[:, c, 0:97], oh[:],
                              start=(r == 0), stop=(r == SEG_PER_BLK - 1),
                              skip_group_check=True, tile_position=(0, 0))
                          if r == SEG_PER_BLK - 1:
                              nc.scalar.activation(
                                  aggT[g][0:97, j * 128:(j + 1) * 128],
                                  ps_blk[g][0:97, :], AF.Copy)

              # per-batch 13-row agg tiles: rows 0:12 agg features, row 12
              # the deg>0 indicator (invdeg-weighted ones-row aggregate).
              # Copied per column-chunk so each dense group only waits for
              # the aggT blocks covering ITS columns, not the full gather.
              for g in "bc":
                  agb_t[g] = [cpool.tile([16, NPCP], f16, tag=f"ag{g}{b}",
                                         name=f"ag{g}{b}")
                              for b in range(B)]
              for (c0, kl) in CHUNKS:
                  for g in "bc":
                      for b in range(B):
                          ag = agb_t[g][b]
                          nc.sync.dma_start(
                              ag[0:12, c0:c0 + kl],
                              aggT[g][12 * b:12 * b + 12, c0:c0 + kl])
                          nc.sync.dma_start(ag[12:13, c0:c0 + kl],
                                            aggT[g][96:97, c0:c0 + kl])

              # ---- dense phase in groups of GRP pairs (chunk-major: each
              # group covers ONE column chunk of all 8 batches, so group 0
              # unlocks as soon as the early aggT blocks are done) ----
              pairs = [(b, c0, kl) for (c0, kl) in CHUNKS for b in range(B)]
              with (
                  tc.tile_pool(name="mainps", bufs=2, space="PSUM") as mainps,
                  tc.tile_pool(name="statps", bufs=2, space="PSUM") as statps,
                  tc.tile_pool(name="ebc", bufs=2, space="PSUM") as ebcps,
                  tc.tile_pool(name="shl", bufs=GRP + 1) as shlpool,
                  tc.tile_pool(name="sr", bufs=GRP + 1) as srpool,
                  tc.tile_pool(name="hla", bufs=GRP + 1) as hlapool,
                  tc.tile_pool(name="sq", bufs=2) as sqpool,
                  tc.tile_pool(name="ssb", bufs=2) as ssbpool,
                  tc.tile_pool(name="stg", bufs=2) as stgpool,
                  tc.tile_pool(name="tmp", bufs=2) as tmppool,
                  tc.tile_pool(name="stat", bufs=2) as statpool,
                  tc.tile_pool(name="smax", bufs=2) as smaxpool,
              ):
               for grp in range(NGRP):
                  gpairs = list(enumerate(pairs[grp * GRP:(grp + 1) * GRP]))
                  # stats tiles in 4-row pair groups so the sumsq psum rows
                  # {32,33,64,65} land with ONE regular-pattern DMA per pair;
                  # st1/st2/aux are separate base-0 tiles (TensorTensor needs
                  # equal input base partitions)
                  st1 = statpool.tile([32, 512], f32, tag="st1")
                  st2 = statpool.tile([32, 512], f32, tag="st2")
                  sdt = smaxpool.tile([GRP, 512], f32, tag="sdt")
                  nc.gpsimd.memset(st1[:], 0.0)
                  nc.gpsimd.memset(st2[:], 1.0)
                  nc.gpsimd.memset(sdt[:], 0.0)
                  shl_t, sr_t, hla_t = {}, {}, {}

                  for q, (b, c0, kl) in gpairs:
                      xr = xb_t[b][0:13, c0:c0 + kl]
                      ab = agb_t['b'][b][0:13, c0:c0 + kl]
                      ac = agb_t['c'][b][0:13, c0:c0 + kl]

                      phl = mainps.tile([128, 512], f32, tag="phl")
                      mmg([(phl[:, 0:kl], w1, xr, (0, 0)),
                           (phl[:, 0:kl], w2, ab, (0, 0)),
                           (phl[64:128, 0:kl], w3, ac, (0, 64))])
                      pres = mainps.tile([64, 512], f32, tag="pres")
                      mmg([(pres[:, 0:kl], wr, xr, (0, 0))])

                      sh = shlpool.tile([128, 512], f16, tag="shl")
                      shl_t[q] = sh
                      nc.scalar.activation(sh[:, 0:kl], phl[:, 0:kl], AF.Copy)
                      sr = srpool.tile([64, 512], f16, tag="sr",
                                       name=f"sr{q}")
                      sr_t[q] = sr
                      nc.scalar.activation(sr[0:64, 0:kl],
                                           pres[:, 0:kl], AF.Copy)
                      sq = sqpool.tile([128, 512], f32, tag="sq")
                      nc.scalar.activation(sq[:, 0:kl], sh[:, 0:kl], AF.Square)
                      sqr = sqpool.tile([64, 512], f32, tag="sqr")
                      nc.scalar.activation(sqr[:, 0:kl],
                                           sr[0:64, 0:kl], AF.Square)

                      # stats psum (already /OD): means@0:3, meansq_hl@32:34,
                      # meansq_r@64:66 (row 65 is a zero column of ones64)
                      S = statps.tile([66, 512], f32, tag="S")
                      mmg([(S[0:3, 0:kl], wsx, xr, (0, 0)),
                           (S[0:2, 0:kl], wsab, ab, (0, 0)),
                           (S[0:2, 0:kl], wsac, ac, (0, 0))])
                      mmg([(S[32:34, 0:kl], oneshl_t[:], sq[:, 0:kl],
                            (0, 32))])
                      mmg([(S[64:66, 0:kl], ones64_t[:], sqr[:, 0:kl],
                            (0, 64))])
                      ssb = ssbpool.tile([96, 512], f32, tag="ssb")
                      nc.scalar.activation(ssb[0:66, 0:kl], S[:, 0:kl],
                                           AF.Copy)
                      nc.sync.dma_start(st1[4 * q:4 * q + 3, 0:kl],
                                        ssb[0:3, 0:kl])
                      nc.sync.dma_start(st2[4 * q:4 * q + 2, 0:kl],
                                        ssb[32:34, 0:kl])
                      nc.sync.dma_start(st2[4 * q + 2:4 * q + 4, 0:kl],
                                        ssb[64:66, 0:kl])

                  # ---- batched stats math (in-place to save SBUF) ----
                  # st1 = means; st2 meansq -> var -> rstd; aux m^2 -> std
                  # -> m*rstd
                  aux = statpool.tile([32, 512], f32, tag="aux")
                  nc.vector.tensor_mul(aux[:], st1[:], st1[:])
                  nc.vector.tensor_sub(st2[:], st2[:], aux[:])
                  nc.scalar.activation(aux[:], st2[:], AF.Sqrt,
                                       bias=eps_t[:])
                  nc.vector.reciprocal(st2[:], aux[:])
                  nc.vector.tensor_mul(aux[:], st1[:], st2[:])
                  rstd, mrstd = st2, aux

                  # ---- per-pair LN apply + activations + logit diff ----
                  for q, (b, c0, kl) in gpairs:
                      sh = shl_t[q]
                      # Pool-issued staging loads offload the SP sequencer
                      rstg = stgpool.tile([2, 512], f32, tag="rstg")
                      nc.gpsimd.dma_start(rstg[:, 0:kl],
                                          rstd[4 * q:4 * q + 2, 0:kl])
                      mstg = stgpool.tile([2, 512], f32, tag="mstg")
                      nc.gpsimd.dma_start(mstg[:, 0:kl],
                                          mrstd[4 * q:4 * q + 2, 0:kl])
                      rbc = ebcps.tile([128, 512], f32, tag="ebc")
                      nc.tensor.matmul(rbc[:, 0:kl], ehl[:],
                                       rstg[:, 0:kl],
                                       start=True, stop=True,
                                       skip_group_check=True,
                                       tile_position=(0, 0))
                      mbc = ebcps.tile([128, 512], f32, tag="ebc")
                      nc.tensor.matmul(mbc[:, 0:kl], ehl[:],
                                       mstg[:, 0:kl],
                                       start=True, stop=True,
                                       skip_group_check=True,
                                       tile_position=(0, 0))
                      t1 = tmppool.tile([128, 512], f32, tag="t1")
                      nc.vector.tensor_mul(t1[:, 0:kl], sh[:, 0:kl],
                                           rbc[:, 0:kl])
                      t2 = tmppool.tile([128, 512], f32, tag="t2")
                      nc.vector.tensor_sub(t2[:, 0:kl], t1[:, 0:kl],
                                           mbc[:, 0:kl])
                      hla = hlapool.tile([128, 512], f32, tag="hla")
                      hla_t[q] = hla
                      yh = tmppool.tile([64, 512], f32, tag="yh")
                      nc.scalar.activation(yh[:, 0:kl], t2[0:64, 0:kl],
                                           AF.Identity,
                                           bias=pv_t[0:64, 1:2],
                                           scale=pv_t[0:64, 0:1])
                      nc.vector.scalar_tensor_tensor(
                          hla[0:64, 0:kl], yh[:, 0:kl], 0.1, yh[:, 0:kl],
                          OP.mult, OP.max)
                      nc.scalar.activation(hla[64:128, 0:kl], t2[64:128, 0:kl],
                                           AF.Gelu,
                                           bias=pv_t[64:128, 1:2],
                                           scale=pv_t[64:128, 0:1])
                      klo = min(kl, NPC - c0)
                      nc.sync.dma_start(out_d.ap()[1, b, :, c0:c0 + klo],
                                        hla[0:64, 0:klo])
                      nc.sync.dma_start(out_d.ap()[2, b, :, c0:c0 + klo],
                                        hla[64:128, 0:klo])
                      sd = statps.tile([1, 512], f32, tag="S")
                      nc.tensor.matmul(sd[:, 0:kl], pv_t[:, 2:3],
                                       hla[:, 0:kl],
                                       start=True, stop=True,
                                       skip_group_check=True,
                                       tile_position=(0, 0))
                      sdb = ssbpool.tile([1, 512], f32, tag="sdb")
                      nc.scalar.activation(sdb[:, 0:kl], sd[:, 0:kl], AF.Copy)
                      nc.sync.dma_start(sdt[q:q + 1, 0:kl], sdb[:, 0:kl])

                  # ---- batched 2-way softmax (in-place to save SBUF) ----
                  a0 = smaxpool.tile([GRP, 512], f32, tag="a0")
                  nc.scalar.activation(a0[:], sdt[:], AF.Sigmoid,
                                       bias=bad_t[:])
                  w0 = sdt
                  nc.vector.tensor_scalar_add(w0[:], a0[:], 0.3)
                  w1_ = a0
                  nc.scalar.activation(w1_[:], a0[:], AF.Identity,
                                       bias=c13_t[:], scale=-1.0)

                  # ---- per-pair fusion + residual + output ----
                  for q, (b, c0, kl) in gpairs:
                      hla = hla_t[q]
                      sr = sr_t[q]
                      w0s = stgpool.tile([1, 512], f32, tag="w0s")
                      nc.sync.dma_start(w0s[:, 0:kl], w0[q:q + 1, 0:kl])
                      w1s = stgpool.tile([1, 512], f32, tag="w1s")
                      nc.sync.dma_start(w1s[:, 0:kl], w1_[q:q + 1, 0:kl])
                      wbc = ebcps.tile([128, 512], f32, tag="ebc")
                      nc.tensor.matmul(wbc[:, 0:kl], ehl[0:1, :],
                                       w0s[:, 0:kl], start=True,
                                       stop=False, skip_group_check=True,
                                       tile_position=(0, 0))
                      nc.tensor.matmul(wbc[:, 0:kl], ec1[:],
                                       w1s[:, 0:kl], start=False,
                                       stop=True, skip_group_check=True,
                                       tile_position=(0, 0))
                      f1 = tmppool.tile([128, 512], f32, tag="f1")
                      nc.vector.tensor_mul(f1[:, 0:kl], hla[:, 0:kl],
                                           wbc[:, 0:kl])
                      rrs = stgpool.tile([1, 512], f32, tag="rrs")
                      nc.gpsimd.dma_start(rrs[:, 0:kl],
                                          rstd[4 * q + 2:4 * q + 3, 0:kl])
                      rms = stgpool.tile([1, 512], f32, tag="rms")
                      nc.gpsimd.dma_start(rms[:, 0:kl],
                                          mrstd[4 * q + 2:4 * q + 3, 0:kl])
                      rr = ebcps.tile([64, 512], f32, tag="ebc")
                      nc.tensor.matmul(rr[:, 0:kl], onesr_t[:],
                                       rrs[:, 0:kl],
                                       start=True, stop=True,
                                       skip_group_check=True,
                                       tile_position=(0, 0))
                      rm = ebcps.tile([64, 512], f32, tag="ebc")
                      nc.tensor.matmul(rm[:, 0:kl], onesr_t[:],
                                       rms[:, 0:kl],
                                       start=True, stop=True,
                                       skip_group_check=True,
                                       tile_position=(0, 0))
                      u1 = tmppool.tile([64, 512], f32, tag="u1")
                      nc.vector.tensor_mul(u1[:, 0:kl], sr[0:64, 0:kl],
                                           rr[:, 0:kl])
                      u2 = tmppool.tile([64, 512], f32, tag="u2")
                      nc.vector.tensor_sub(u2[:, 0:kl], u1[:, 0:kl],
                                           rm[:, 0:kl])
                      resa = tmppool.tile([64, 512], f32, tag="resa")
                      nc.scalar.activation(resa[:, 0:kl], u2[:, 0:kl],
                                           AF.Identity,
                                           bias=pv_t[0:64, 4:5],
                                           scale=pv_t[0:64, 3:4])
                      f2 = ebcps.tile([64, 512], f32, tag="ebc")
                      nc.tensor.matmul(f2[:, 0:kl], ident2[:], f1[:, 0:kl],
                                       start=True, stop=True,
                                       skip_group_check=True,
                                       tile_position=(0, 0))
                      f3 = tmppool.tile([64, 512], f32, tag="f3")
                      nc.vector.tensor_add(f3[:, 0:kl], f2[:, 0:kl],
                                           resa[:, 0:kl])
                      klo = min(kl, NPC - c0)
                      nc.sync.dma_start(out_d.ap()[0, b, :, c0:c0 + klo],
                                        f3[:, 0:klo])
    nc.finalize()
    return nc


# ------------------------------------------------------------------- runner
class _SpmdRunner:
    def __init__(self, nc, n_cores=NCORE):
        import jax
        from jax.sharding import Mesh, PartitionSpec
        from jax.experimental.shard_map import shard_map
        from concourse import mybir
        from concourse.bass2jax import (_bass_exec_p, install_neuronx_cc_hook,
                                        partition_id_tensor)
        install_neuronx_cc_hook()
        self.jax = jax
        self.n_cores = n_cores
        partition_name = (nc.partition_id_tensor.name
                          if nc.partition_id_tensor else None)
        in_names, out_names, out_avals = [], [], []
        for alloc in nc.m.functions[0].allocations:
            if not isinstance(alloc, mybir.MemoryLocationSet):
                continue
            name = alloc.memorylocations[0].name
            if alloc.kind == "ExternalInput":
                if name != partition_name:
                    in_names.append(name)
            elif alloc.kind == "ExternalOutput":
                out_names.append(name)
                shape = tuple(alloc.tensor_shape)
                dtype = mybir.dt.np(alloc.dtype)
                out_avals.append(jax.core.ShapedArray(shape, dtype))
        self.in_names, self.out_names = in_names, out_names
        self.out_avals = out_avals
        n_params = len(in_names)
        # The kernel writes every element of every output, so the pre-zeroed
        # output operands of the stock runner are dropped — they would be
        # re-streamed to the terminal on every execute.
        all_in = list(in_names)
        if partition_name is not None:
            all_in.append(partition_name)

        def _body(*args):
            operands = list(args)
            if partition_name is not None:
                operands.append(partition_id_tensor())
            outs = _bass_exec_p.bind(
                *operands, out_avals=tuple(out_avals),
                in_names=tuple(all_in), out_names=tuple(out_names),
                lowering_input_output_aliases=(),
                sim_require_finite=True, sim_require_nnan=True, nc=nc)
            return tuple(outs)

        devices = jax.devices()[:n_cores]
        mesh = Mesh(np.asarray(devices), ("core",))
        in_specs = (PartitionSpec("core"),) * n_params
        out_specs = (PartitionSpec("core"),) * len(out_names)
        self.fn = jax.jit(
            shard_map(_body, mesh=mesh, in_specs=in_specs,
                      out_specs=out_specs, check_rep=False),
            keep_unused=True)

    def prepare(self, in_maps):
        n = self.n_cores
        per_core = [[np.ascontiguousarray(m[name]) for name in self.in_names]
                    for m in in_maps]
        concat_in = [np.concatenate([per_core[c][i] for c in range(n)], axis=0)
                     for i in range(len(self.in_names))]
        return [self.jax.device_put(a) for a in concat_in]

    def run(self, args):
        outs = self.fn(*args)
        self.jax.block_until_ready(outs)
        return outs

    def split_outs(self, outs):
        res = []
        for c in range(self.n_cores):
            d = {}
            for i, name in enumerate(self.out_names):
                d[name] = np.asarray(outs[i]).reshape(
                    self.n_cores, *self.out_avals[i].shape)[c]
            res.append(d)
        return res


# -------------------------------------------------------------------- entry
def _get(inputs):
    gb = _prep_graph(inputs['edge_index'])
    gc = _prep_graph(inputs['causal_edge_index'])
    b_pad = max(128, -(-int(max(gb['counts'].max(), gc['counts'].max()))
                     // 128) * 128)
    stream = NBLK * b_pad
    nseg = stream // 128
    nch = -(-stream // 1024)
    pw, pv, ba_diff = _pack_weights(inputs)
    key = (b_pad, nseg, nch, round(ba_diff, 9), REPEAT,
           NQ_ALLOC, NQ_SPREAD, XTS_SHARED)
    if key not in _cache:
        nc = _build_program(nseg, nch, ba_diff, REPEAT)
        _cache[key] = _SpmdRunner(nc)
    return _cache[key], gb, gc, b_pad, nseg, nch, pw, pv


def make_in_maps(inputs):
    runner, gb, gc, b_pad, nseg, nch, pw, pv = _get(inputs)
    x = np.asarray(inputs['x'], np.float32)
    xflat = x.reshape(96, N)
    idx_b, dli_b = _build_streams(gb, b_pad, nseg, nch)
    idx_c, dli_c = _build_streams(gc, b_pad, nseg, nch)
    h = lambda a: np.ascontiguousarray(a).view(np.float16).ravel()
    in_maps = []
    for c in range(NCORE):
        xsh = np.zeros((96, NPCP), np.float16)
        xsh[:, 0:NPC] = xflat[:, c * NPC:(c + 1) * NPC]
        blob = np.concatenate([
            h(xsh), h(pw), h(pv),
            h(idx_b[c]), h(idx_c[c]),
            h(dli_b[c]), h(dli_c[c]),
        ])[None, :]
        in_maps.append({'blob': blob})
    return runner, in_maps


def kernel(**inputs):
    runner, in_maps = make_in_maps(inputs)
    args = runner.prepare(in_maps)
    outs = runner.run(args)
    res = runner.split_outs(outs)
    fused = np.empty((B, OD, N), np.float32)
    high = np.empty((B, OD, N), np.float32)
    low = np.empty((B, OD, N), np.float32)
    for c in range(NCORE):
        sl = slice(c * NPC, (c + 1) * NPC)
        o = res[c]['out'].astype(np.float32)
        fused[:, :, sl] = o[0]
        high[:, :, sl] = o[1]
        low[:, :, sl] = o[2]
    return fused, high, low



# revision 56
# speedup vs baseline: 1.7407x; 1.5790x over previous
"""Trainium2 Bass kernel for nn_DWTEnhancedSTGCN (B=8, T=12, N=10000, E=160000).

Strategy (N-sharded over 8 NeuronCores), I/O-minimized:
  - The axon tunnel re-streams every input (and the pre-zeroed output
    buffers) on each execute, and on-device compute is ~free, so the design
    minimizes per-call bytes: fp16 node features / index payloads / outputs,
    weights packed to one 13-row copy (the per-batch [128,128] blocks of the
    old layout all held identical content), structural constants (identity
    blocks, half-selectors, ones) generated on device with iota/memset, the
    pre-zeroed output operands dropped (every output element is written),
    and the full-graph gather table built ON DEVICE: each core ships only
    its own x slice, PE-transposes it locally, and an AllGather of the
    transposed stripes assembles the [node, feature] fp16 table in HBM.
    Outputs are written f32 (D2H fetch is untimed; dropping the f16
    down-convert relieves the ACT engine). Gathers spread over 4 SWDGE
    queues; dense-phase pools are deep enough to pipeline across groups;
    stats staging loads are Pool-issued to offload the SP sequencer, and
    the LN stats matmul weights are pre-divided by OD so means come
    straight off the PE. Stats tiles use 4-row pair groups so the sumsq
    psum rows land in two contiguous DMAs.
  - Each core owns 1250 dst-nodes for ALL 8 batch elements; each edge's 96
    batch-features (+ones) are gathered once per core via dma_gather (256B
    fp16 rows from the on-device table).
  - Aggregation = mean over in-edges, folded into one-hot segment-sum
    matmuls on the PE: onehot[e, j] = (iota[j] == dstloc[e]) * invdeg
    (one fused DVE tensor_scalar), then aggT_block += G_chunk.T @ onehot.
  - Dense phase in [feature, node] layout; per-batch 13-row rhs tiles
    (12 x-features + ones/indicator row) contract against the packed
    weights. LayerNorm stats via ones-matmul column sums, batched across
    node-chunks; activations on ACT; fusion on DVE.
Host does only: sharding/reshapes, integer index-stream building, and
parameter-only weight folding. All FP math on x runs on device.
"""
import sys
import numpy as np

sys.path.insert(0, '/opt/trn_rl_repo')

B, T, N = 8, 12, 10000
OD = 64
NCORE = 8
NPC = N // NCORE          # 1250 nodes per core
NPCP = 1280               # padded local node count (10 blocks of 128)
NBLK = NPCP // 128
EPS = 1e-5
PADROW = NPCP - 2         # all-zero pad row (stripe-0 tail) for padding idxs
NROWS = NCORE * NPCP      # table rows: per-core 1280-row stripes
CHUNKS = [(0, 512), (512, 512), (1024, 256)]
NPAIR = B * len(CHUNKS)   # 24
GRP = 4                   # pairs per stats/softmax group
NGRP = NPAIR // GRP
REPEAT = 1                # in-kernel repetition (timing mode)
NQ_ALLOC = 4              # SWDGE queues allocated (1..4)
NQ_SPREAD = 4             # queues the gathers round-robin over (<= NQ_ALLOC)
XTS_SHARED = True         # AllGather output in Shared addr space

# packed-weight column layout: [16, PWC] fp16; rows 0:12 = weight rows,
# row 12 = bias row (pairs with the ones/indicator row of the rhs tiles)
PW = {'w1': slice(0, 128), 'w2': slice(128, 256), 'w3': slice(256, 320),
      'wr': slice(320, 384), 'wsx': slice(384, 387), 'wsab': slice(387, 389),
      'wsac': slice(389, 391)}
PWC = 391

_cache = {}


# ----------------------------------------------------------------- host prep
def _prep_graph(edge_index):
    src = np.asarray(edge_index[0]).astype(np.int64).ravel()
    dst = np.asarray(edge_index[1]).astype(np.int64).ravel()
    deg = np.bincount(dst, minlength=N)
    invdeg = (1.0 / np.maximum(deg, 1)).astype(np.float32)
    order = np.argsort(dst, kind='stable')
    s_s, d_s = src[order], dst[order]
    core = d_s // NPC
    local = d_s - core * NPC
    blk = local >> 7
    dstloc = local & 127
    binid = core * NBLK + blk
    counts = np.bincount(binid, minlength=NCORE * NBLK)
    return dict(s=s_s, d=d_s, core=core, binid=binid, dstloc=dstloc,
                blk=blk, counts=counts, invdeg=invdeg)


def _build_streams(g, b_pad, nseg, nch):
    stream = NBLK * b_pad
    starts = np.zeros(NCORE * NBLK, np.int64)
    np.cumsum(g['counts'][:-1], out=starts[1:])
    rank = np.arange(len(g['s'])) - starts[g['binid']]
    pos = g['core'] * stream + g['blk'] * b_pad + rank
    src_stream = np.full(NCORE * stream, PADROW, np.int64)
    dl_stream = np.zeros(NCORE * stream, np.float32)
    iv_stream = np.zeros(NCORE * stream, np.float32)
    # table rows are per-core 1280-row stripes: row = core*NPCP + local
    src_stream[pos] = (g['s'] // NPC) * NPCP + (g['s'] % NPC)
    dl_stream[pos] = g['dstloc']
    iv_stream[pos] = g['invdeg'][g['d']]
    idxs, dlis = [], []
    for c in range(NCORE):
        st = src_stream[c * stream:(c + 1) * stream]
        stp = np.full(nch * 1024, PADROW, np.int64)
        stp[:stream] = st
        t16 = stp.reshape(nch, 64, 16).transpose(2, 0, 1).reshape(16, nch * 64)
        idxs.append(np.ascontiguousarray(t16.astype(np.int16)))
        dl = dl_stream[c * stream:(c + 1) * stream].reshape(nseg, 128).T
        iv = iv_stream[c * stream:(c + 1) * stream].reshape(nseg, 128).T
        dlis.append(np.ascontiguousarray(
            np.concatenate([dl, iv], axis=1).astype(np.float16)))
    return idxs, dlis


def _pack_weights(p):
    f = lambda k: np.asarray(p[k], np.float32)
    h16 = lambda a: a.astype(np.float16)
    W_ht, b_ht, W_lt, b_lt = f('W_ht'), f('b_ht'), f('W_lt'), f('b_lt')
    Ws_h, Wn_h, b_h = f('Ws_h'), f('Wn_h'), f('b_h')
    Ws_l, Wn_l, Wc_l, b_l = f('Ws_l'), f('Wn_l'), f('Wc_l'), f('b_l')
    Whr, bhr, Wlr, blr = f('Whr'), f('bhr'), f('Wlr'), f('blr')
    Wg, bg = f('Wg'), f('bg')
    W1 = h16(np.concatenate([W_ht @ (Ws_h + 0.2 * Whr),
                             W_lt @ (Ws_l + 0.2 * Wlr)], 1))
    W1b = h16(np.concatenate([b_ht @ (Ws_h + 0.2 * Whr) + b_h + 0.2 * bhr,
                              b_lt @ (Ws_l + 0.2 * Wlr) + b_l + 0.2 * blr]))
    W2 = h16(np.concatenate([W_ht @ Wn_h, W_lt @ Wn_l], 1))
    W2b = h16(np.concatenate([b_ht @ Wn_h, b_lt @ Wn_l]))
    W3 = h16(W_lt @ Wc_l)
    W3b = h16(b_lt @ Wc_l)
    WR = h16(2.0 * Wg)
    WRb = h16(bg)
    # column sums of the f16-rounded matrices, so the on-device mean matches
    # the f16 matmul results up to one extra rounding
    s32 = lambda a: a.astype(np.float32)
    pw = np.zeros((16, PWC), np.float16)
    pw[0:12, PW['w1']], pw[12, PW['w1']] = W1, W1b
    pw[0:12, PW['w2']], pw[12, PW['w2']] = W2, W2b
    pw[0:12, PW['w3']], pw[12, PW['w3']] = W3, W3b
    pw[0:12, PW['wr']], pw[12, PW['wr']] = WR, WRb
    # stats weights pre-divided by OD so the PE emits means directly
    r = 1.0 / OD
    pw[0:12, PW['wsx']] = h16(r * np.stack(
        [s32(W1[:, 0:64]).sum(1), s32(W1[:, 64:128]).sum(1),
         s32(WR).sum(1)], 1))
    pw[12, PW['wsx']] = h16(r * np.array(
        [s32(W1b[0:64]).sum(), s32(W1b[64:128]).sum(), s32(WRb).sum()]))
    pw[0:12, PW['wsab']] = h16(r * np.stack(
        [s32(W2[:, 0:64]).sum(1), s32(W2[:, 64:128]).sum(1)], 1))
    pw[12, PW['wsab']] = h16(r * np.array(
        [s32(W2b[0:64]).sum(), s32(W2b[64:128]).sum()]))
    pw[0:12, PW['wsac']] = h16(r * np.stack(
        [np.zeros(T, np.float32), s32(W3).sum(1)], 1))
    pw[12, PW['wsac']] = h16(r * np.array([0.0, s32(W3b).sum()]))

    pv = np.zeros((128, 8), np.float32)
    pv[:, 0] = np.concatenate([f('g_hn'), f('g_ln')])
    pv[:, 1] = np.concatenate([f('b_hn'), f('b_ln')])
    pv[:, 2] = f('Wa')[:, 0] - f('Wa')[:, 1]
    pv[0:64, 3] = 0.1 * f('g_gn')
    pv[0:64, 4] = 0.1 * f('b_gn')
    ba = f('ba')
    return pw, pv, float(ba[0] - ba[1])


# -------------------------------------------------------------- bass program
def _build_program(nseg, nch, ba_diff, repeat=1):
    import concourse.tile as tile
    from concourse import bacc, mybir

    f32 = mybir.dt.float32
    f16 = mybir.dt.float16
    i16 = mybir.dt.int16
    AF = mybir.ActivationFunctionType
    OP = mybir.AluOpType
    SEG_PER_BLK = nseg // NBLK

    nc = bacc.Bacc("TRN2", target_bir_lowering=False, debug=False,
                   enable_asserts=False, num_devices=NCORE,
                   num_swdge_queues=NQ_ALLOC)

    # single packed input buffer per core (fewer PJRT buffers = less
    # execute-path jitter); sections are f16-viewed flat byte ranges
    off = {}
    _o = 0
    for name, n in [('xsh', 96 * NPCP), ('pw', 16 * PWC), ('pv', 128 * 16),
                    ('idx_b', 16 * nch * 64), ('idx_c', 16 * nch * 64),
                    ('dli_b', 128 * 2 * nseg), ('dli_c', 128 * 2 * nseg)]:
        off[name] = (_o, n)
        _o += n
    BLOB = _o
    blob_d = nc.dram_tensor("blob", [1, BLOB], f16, kind="ExternalInput")

    def sect(name, dt, cols):
        o, n = off[name]
        ap = blob_d.ap()[0:1, o:o + n]
        if dt is not f16:
            ap = ap.bitcast(dt)
        return ap.rearrange("a (r c) -> (a r) c", c=cols)

    xsh_ap = sect('xsh', f16, NPCP)
    pw_ap = sect('pw', f16, PWC)
    pv_ap = sect('pv', f32, 8)
    idx_ap = {g: sect(f'idx_{g}', i16, nch * 64) for g in "bc"}
    dli_ap = {g: sect(f'dli_{g}', f16, 2 * nseg) for g in "bc"}
    # gather table built on device: PE-transpose the OWN x slice into
    # [node, feature] rows, then AllGather the per-core transposed stripes
    # straight into the final table (per-core 1280-row stripes; pad rows
    # are zero because the x slice's pad columns are host-zeroed).
    xtsl_d = nc.dram_tensor("xtsl", [NPCP, 128], f16, kind="Internal")
    xts_d = nc.dram_tensor("xts", [NROWS, 128], f16, kind="Internal",
                           addr_space="Shared" if XTS_SHARED else "Local")
    out_d = nc.dram_tensor("out", [3, B, OD, NPC], f32, kind="ExternalOutput")

    def mmg(mms):
        """Emit matmuls as one PSUM accumulation group.
        mms: list of (out_ap, lhsT_ap, rhs_ap, tile_position)."""
        nmm = len(mms)
        for i, (out, lhsT, rhs, tp) in enumerate(mms):
            nc.tensor.matmul(out, lhsT, rhs, start=(i == 0),
                             stop=(i == nmm - 1), skip_group_check=True,
                             tile_position=tp)

    with tile.TileContext(nc) as tc:
        with (
            tc.tile_pool(name="const", bufs=1) as cpool,
        ):
            pw_t = cpool.tile([16, PWC], f16, tag="pw")
            nc.sync.dma_start(pw_t[:], pw_ap)
            pv_t = cpool.tile([128, 8], f32, tag="pv")
            nc.sync.dma_start(pv_t[:], pv_ap)

            w1 = pw_t[0:13, PW['w1']]
            w2 = pw_t[0:13, PW['w2']]
            w3 = pw_t[0:13, PW['w3']]
            wr = pw_t[0:13, PW['wr']]
            wsx = pw_t[0:13, PW['wsx']]
            wsab = pw_t[0:13, PW['wsab']]
            wsac = pw_t[0:13, PW['wsac']]

            # structural constants, generated on device (single-partition
            # rows built at partition 0, then DMA'd into place — compute
            # engines cannot start at unaligned partitions)
            ec1 = cpool.tile([1, 128], f32, tag="ec1")
            nc.gpsimd.memset(ec1[:, 0:64], 0.0)
            nc.gpsimd.memset(ec1[:, 64:128], 1.0)
            # copy of ec1 at partition 32: pairs with the w1 row of the
            # merged softmax tile (PE needs equal lhsT/rhs base partitions)
            ec132 = cpool.tile([33, 128], f32, tag="ec132")
            nc.gpsimd.memset(ec132[32:33, 0:64], 0.0)
            nc.gpsimd.memset(ec132[32:33, 64:128], 1.0)
            ehl = cpool.tile([2, 128], f32, tag="ehl")
            nc.sync.dma_start(ehl[1:2, :], ec1[:])
            onesr_t = cpool.tile([1, 64], f32, tag="onesr")
            nc.gpsimd.memset(onesr_t[:], 1.0)
            # sumsq reducers pre-scaled by 1/OD (stats arrive as means);
            # ones64 col 1 is zero so the r-sumsq occupies rows 64:66 of the
            # stats psum and the {32,33,64,65} rows form one regular pattern
            ones64_t = cpool.tile([64, 2], f32, tag="ones64")
            nc.gpsimd.memset(ones64_t[:], 0.0)
            nc.gpsimd.memset(ones64_t[:, 0:1], 1.0 / OD)
            oneshl_t = cpool.tile([128, 2], f32, tag="oneshl")
            nc.gpsimd.memset(oneshl_t[:], 0.0)
            nc.gpsimd.memset(oneshl_t[0:64, 0:1], 1.0 / OD)
            nc.gpsimd.memset(oneshl_t[64:128, 1:2], 1.0 / OD)
            ident2 = cpool.tile([128, 64], f32, tag="ident2")
            eps_t = cpool.tile([32, 1], f32, tag="eps")
            nc.gpsimd.memset(eps_t[:], EPS)
            bad_t = cpool.tile([GRP, 1], f32, tag="bad")
            nc.gpsimd.memset(bad_t[:], ba_diff)
            c13_t = cpool.tile([GRP, 1], f32, tag="c13")
            nc.gpsimd.memset(c13_t[:], 1.3)

            # per-batch 13-row x tiles: rows 0:12 features, row 12 ones
            xb_t = [cpool.tile([16, NPCP], f16, tag=f"xb{b}", name=f"xb{b}")
                    for b in range(B)]
            aggT = {g: cpool.tile([128, NPCP], f16, tag=f"agg{g}",
                                  name=f"aggT{g}") for g in "bc"}
            for g in "bc":
                nc.gpsimd.memset(aggT[g][:], 0.0)

            # ---- gather + one-hot segment-sum (per graph) ----
            agb_t = {'b': [], 'c': []}
            for _rep in range(repeat):
              with (
                  tc.tile_pool(name="ld", bufs=1) as ldpool,
                  tc.tile_pool(name="gat", bufs=4) as gpool,
                  tc.tile_pool(name="oh", bufs=8) as ohpool,
                  tc.tile_pool(name="aggps", bufs=4, space="PSUM") as aggps,
              ):
                # gather-scoped loads + on-device constant builds (the pool
                # frees before the dense-phase pools open)
                xsh_t = ldpool.tile([96, NPCP], f16, tag="xsh")
                nc.sync.dma_start(xsh_t[:], xsh_ap)
                ones_h = ldpool.tile([1, NPCP], f16, tag="onesh")
                nc.gpsimd.memset(ones_h[:], 1.0)
                ec0 = ldpool.tile([1, 128], f32, tag="ec0")
                nc.gpsimd.memset(ec0[:, 0:64], 1.0)
                nc.gpsimd.memset(ec0[:, 64:128], 0.0)
                nc.sync.dma_start(ehl[0:1, :], ec0[:])
                for b in range(B):
                    nc.sync.dma_start(xb_t[b][0:12, :],
                                      xsh_t[12 * b:12 * b + 12, :])
                    nc.sync.dma_start(xb_t[b][12:13, :], ones_h[:])
                iota_h = ldpool.tile([128, 128], f16, tag="iotah")
                nc.gpsimd.iota(iota_h[:], pattern=[[1, 128]], base=0,
                               channel_multiplier=0,
                               allow_small_or_imprecise_dtypes=True)
                ic_t = ldpool.tile([128, 64], f32, tag="ic")
                nc.gpsimd.iota(ic_t[:], pattern=[[1, 64]], base=0,
                               channel_multiplier=0,
                               allow_small_or_imprecise_dtypes=True)
                ip_t = ldpool.tile([128, 1], f32, tag="ip")
                nc.gpsimd.iota(ip_t[:], pattern=[[1, 1]], base=0,
                               channel_multiplier=1,
                               allow_small_or_imprecise_dtypes=True)
                ige_t = ldpool.tile([128, 1], f32, tag="ige")
                nc.vector.tensor_scalar(ige_t[:], ip_t[:], 64.0, None,
                                        OP.is_ge)
                ipm_t = ldpool.tile([128, 1], f32, tag="ipm")
                nc.vector.scalar_tensor_tensor(ipm_t[:], ige_t[:], -64.0,
                                               ip_t[:], OP.mult, OP.add)
                nc.vector.tensor_scalar(ident2[:], ic_t[:], ipm_t[:], None,
                                        OP.is_equal)
                idx_t, dli_t = {}, {}
                for g in "bc":
                    idx_t[g] = ldpool.tile([128, nch * 64], i16,
                                           tag=f"idx{g}", name=f"idx{g}")
                    for c in range(8):
                        nc.sync.dma_start(idx_t[g][16 * c:16 * c + 16, :],
                                          idx_ap[g])
                    dli16 = ldpool.tile([128, 2 * nseg], f16,
                                        tag=f"dli16{g}", name=f"dli16{g}")
                    nc.sync.dma_start(dli16[:], dli_ap[g])
                    dli_t[g] = ldpool.tile([128, 2 * nseg], f32,
                                           tag=f"dli{g}", name=f"dli{g}")
                    nc.scalar.activation(dli_t[g][:], dli16[:], AF.Copy)

                # ---- build the gather table on device ----
                # transpose the OWN slab locally, then AllGather the
                # transposed [1280, 128] stripes straight into the table
                ide16 = ldpool.tile([128, 128], f16, tag="ide16")
                nc.vector.tensor_scalar(ide16[:], iota_h[:], ip_t[:], None,
                                        OP.is_equal)
                slab = ldpool.tile([128, NPCP], f16, tag="slab")
                nc.gpsimd.memset(slab[:], 0.0)
                nc.sync.dma_start(slab[0:96, :], xsh_t[:])
                nc.sync.dma_start(slab[96:97, :], ones_h[:])
                with (
                    tc.tile_pool(name="tb", bufs=4) as tbpool,
                    tc.tile_pool(name="tps", bufs=4, space="PSUM") as tpps,
                ):
                    for k in range(NBLK):
                        pst = tpps.tile([128, 128], f16, tag="pst")
                        nc.tensor.transpose(
                            pst[:], slab[:, 128 * k:128 * k + 128],
                            ide16[:])
                        tsb = tbpool.tile([128, 128], f16, tag="tsb")
                        nc.scalar.activation(tsb[:], pst[:], AF.Copy)
                        nc.sync.dma_start(
                            xtsl_d.ap()[128 * k:128 * k + 128, :], tsb[:])
                tc.strict_bb_all_engine_barrier()
                nc.gpsimd.collective_compute(
                    "AllGather", mybir.AluOpType.bypass,
                    replica_groups=[list(range(NCORE))],
                    ins=[xtsl_d.ap()], outs=[xts_d.ap()])
                tc.strict_bb_all_engine_barrier()

                # graphs interleaved chunk-by-chunk so BOTH graphs' dst
                # blocks complete progressively and the dense phase can
                # start on early columns while late chunks still gather
                ps_blk = {}
                for k in range(nch):
                  for gi, g in enumerate("bc"):
                      gt = gpool.tile([128, 8 * 128], f16, tag="g")
                      gt3 = gt[:].rearrange("p (c e) -> p c e", e=128)
                      nc.gpsimd.dma_gather(
                          gt3, xts_d.ap(),
                          idx_t[g][:, k * 64:(k + 1) * 64],
                          num_idxs=1024, num_idxs_reg=1024, elem_size=128,
                          queue_num=(2 * k + gi) % NQ_SPREAD)
                      for c in range(8):
                          s = k * 8 + c
                          if s >= nseg:
                              break
                          r = s % SEG_PER_BLK
                          j = s // SEG_PER_BLK
                          if r == 0:
                              ps_blk[g] = aggps.tile([128, 128], f32,
                                                     tag="agg",
                                                     name=f"agg{g}{j}")
                          oh = ohpool.tile([128, 128], f16, tag="oh")
                          nc.vector.tensor_scalar(
                              oh[:], iota_h[:],
                              dli_t[g][:, s:s + 1],
                              dli_t[g][:, nseg + s:nseg + s + 1],
                              OP.is_equal, OP.mult)
                          nc.tensor.matmul(
                              ps_blk[g][0:97, :], gt3[:, c, 0:97], oh[:],
                              start=(r == 0), stop=(r == SEG_PER_BLK - 1),
                              skip_group_check=True, tile_position=(0, 0))
                          if r == SEG_PER_BLK - 1:
                              nc.scalar.activation(
                                  aggT[g][0:97, j * 128:(j + 1) * 128],
                                  ps_blk[g][0:97, :], AF.Copy)

              # per-batch 13-row agg tiles: rows 0:12 agg features, row 12
              # the deg>0 indicator (invdeg-weighted ones-row aggregate).
              # Copied per column-chunk so each dense group only waits for
              # the aggT blocks covering ITS columns, not the full gather.
              for g in "bc":
                  agb_t[g] = [cpool.tile([16, NPCP], f16, tag=f"ag{g}{b}",
                                         name=f"ag{g}{b}")
                              for b in range(B)]
              for (c0, kl) in CHUNKS:
                  for g in "bc":
                      for b in range(B):
                          ag = agb_t[g][b]
                          nc.sync.dma_start(
                              ag[0:12, c0:c0 + kl],
                              aggT[g][12 * b:12 * b + 12, c0:c0 + kl])
                          nc.sync.dma_start(ag[12:13, c0:c0 + kl],
                                            aggT[g][96:97, c0:c0 + kl])

              # ---- dense phase in groups of GRP pairs (chunk-major: each
              # group covers ONE column chunk of all 8 batches, so group 0
              # unlocks as soon as the early aggT blocks are done) ----
              pairs = [(b, c0, kl) for (c0, kl) in CHUNKS for b in range(B)]
              with (
                  tc.tile_pool(name="mainps", bufs=2, space="PSUM") as mainps,
                  tc.tile_pool(name="statps", bufs=2, space="PSUM") as statps,
                  tc.tile_pool(name="ebc", bufs=2, space="PSUM") as ebcps,
                  tc.tile_pool(name="shl", bufs=GRP + 4) as shlpool,
                  tc.tile_pool(name="sr", bufs=GRP + 4) as srpool,
                  tc.tile_pool(name="hla", bufs=GRP + 4) as hlapool,
                  tc.tile_pool(name="sq", bufs=2) as sqpool,
                  tc.tile_pool(name="ssb", bufs=2) as ssbpool,
                  tc.tile_pool(name="stg", bufs=2) as stgpool,
                  tc.tile_pool(name="tmp", bufs=2) as tmppool,
                  tc.tile_pool(name="stat", bufs=2) as statpool,
                  tc.tile_pool(name="smax", bufs=2) as smaxpool,
              ):
               for grp in range(NGRP):
                  gpairs = list(enumerate(pairs[grp * GRP:(grp + 1) * GRP]))
                  # stats tiles in 4-row pair groups so the sumsq psum rows
                  # {32,33,64,65} land with ONE regular-pattern DMA per pair;
                  # st1/st2/aux are separate base-0 tiles (TensorTensor needs
                  # equal input base partitions)
                  st1 = statpool.tile([32, 512], f32, tag="st1")
                  st2 = statpool.tile([32, 512], f32, tag="st2")
                  sdt = smaxpool.tile([GRP, 512], f32, tag="sdt")
                  nc.gpsimd.memset(st1[:], 0.0)
                  nc.gpsimd.memset(st2[:], 1.0)
                  nc.gpsimd.memset(sdt[:], 0.0)
                  shl_t, sr_t, hla_t = {}, {}, {}

                  for q, (b, c0, kl) in gpairs:
                      xr = xb_t[b][0:13, c0:c0 + kl]
                      ab = agb_t['b'][b][0:13, c0:c0 + kl]
                      ac = agb_t['c'][b][0:13, c0:c0 + kl]

                      phl = mainps.tile([128, 512], f32, tag="phl")
                      mmg([(phl[:, 0:kl], w1, xr, (0, 0)),
                           (phl[:, 0:kl], w2, ab, (0, 0)),
                           (phl[64:128, 0:kl], w3, ac, (0, 64))])
                      pres = mainps.tile([64, 512], f32, tag="pres",
                                         bufs=1)
                      mmg([(pres[:, 0:kl], wr, xr, (0, 0))])

                      sh = shlpool.tile([128, 512], f16, tag="shl")
                      shl_t[q] = sh
                      nc.scalar.activation(sh[:, 0:kl], phl[:, 0:kl], AF.Copy)
                      sr = srpool.tile([64, 512], f16, tag="sr",
                                       name=f"sr{q}")
                      sr_t[q] = sr
                      nc.scalar.activation(sr[0:64, 0:kl],
                                           pres[:, 0:kl], AF.Copy)
                      sq = sqpool.tile([128, 512], f32, tag="sq")
                      nc.scalar.activation(sq[:, 0:kl], sh[:, 0:kl], AF.Square)
                      sqr = sqpool.tile([64, 512], f32, tag="sqr")
                      nc.scalar.activation(sqr[:, 0:kl],
                                           sr[0:64, 0:kl], AF.Square)

                      # stats psum (already /OD): means@0:3, meansq_hl@32:34,
                      # meansq_r@64:66 (row 65 is a zero column of ones64)
                      S = statps.tile([66, 512], f32, tag="S",
                                      bufs=1)
                      mmg([(S[0:3, 0:kl], wsx, xr, (0, 0)),
                           (S[0:2, 0:kl], wsab, ab, (0, 0)),
                           (S[0:2, 0:kl], wsac, ac, (0, 0))])
                      mmg([(S[32:34, 0:kl], oneshl_t[:], sq[:, 0:kl],
                            (0, 32))])
                      mmg([(S[64:66, 0:kl], ones64_t[:], sqr[:, 0:kl],
                            (0, 64))])
                      ssb = ssbpool.tile([96, 512], f32, tag="ssb")
                      nc.scalar.activation(ssb[0:66, 0:kl], S[:, 0:kl],
                                           AF.Copy)
                      nc.sync.dma_start(st1[4 * q:4 * q + 3, 0:kl],
                                        ssb[0:3, 0:kl])
                      nc.sync.dma_start(st2[4 * q:4 * q + 2, 0:kl],
                                        ssb[32:34, 0:kl])
                      nc.sync.dma_start(st2[4 * q + 2:4 * q + 4, 0:kl],
                                        ssb[64:66, 0:kl])

                  # ---- batched stats math (in-place to save SBUF) ----
                  # st1 = means; st2 meansq -> var -> rstd; aux m^2 -> std
                  # -> m*rstd
                  aux = statpool.tile([32, 512], f32, tag="aux")
                  nc.vector.tensor_mul(aux[:], st1[:], st1[:])
                  nc.vector.tensor_sub(st2[:], st2[:], aux[:])
                  nc.scalar.activation(aux[:], st2[:], AF.Sqrt,
                                       bias=eps_t[:])
                  nc.vector.reciprocal(st2[:], aux[:])
                  nc.vector.tensor_mul(aux[:], st1[:], st2[:])
                  rstd, mrstd = st2, aux

                  # ---- per-pair LN apply + activations + logit diff ----
                  for q, (b, c0, kl) in gpairs:
                      sh = shl_t[q]
                      # Pool-issued staging loads offload the SP sequencer
                      rstg = stgpool.tile([2, 512], f32, tag="rstg")
                      nc.gpsimd.dma_start(rstg[:, 0:kl],
                                          rstd[4 * q:4 * q + 2, 0:kl])
                      mstg = stgpool.tile([2, 512], f32, tag="mstg")
                      nc.gpsimd.dma_start(mstg[:, 0:kl],
                                          mrstd[4 * q:4 * q + 2, 0:kl])
                      rbc = ebcps.tile([128, 512], f32, tag="ebc")
                      nc.tensor.matmul(rbc[:, 0:kl], ehl[:],
                                       rstg[:, 0:kl],
                                       start=True, stop=True,
                                       skip_group_check=True,
                                       tile_position=(0, 0))
                      mbc = ebcps.tile([128, 512], f32, tag="ebc")
                      nc.tensor.matmul(mbc[:, 0:kl], ehl[:],
                                       mstg[:, 0:kl],
                                       start=True, stop=True,
                                       skip_group_check=True,
                                       tile_position=(0, 0))
                      t1 = tmppool.tile([128, 512], f32, tag="t1")
                      nc.vector.tensor_mul(t1[:, 0:kl], sh[:, 0:kl],
                                           rbc[:, 0:kl])
                      t2 = tmppool.tile([128, 512], f32, tag="t2")
                      nc.vector.tensor_sub(t2[:, 0:kl], t1[:, 0:kl],
                                           mbc[:, 0:kl])
                      hla = hlapool.tile([128, 512], f32, tag="hla")
                      hla_t[q] = hla
                      yh = tmppool.tile([64, 512], f32, tag="yh")
                      nc.scalar.activation(yh[:, 0:kl], t2[0:64, 0:kl],
                                           AF.Identity,
                                           bias=pv_t[0:64, 1:2],
                                           scale=pv_t[0:64, 0:1])
                      nc.vector.scalar_tensor_tensor(
                          hla[0:64, 0:kl], yh[:, 0:kl], 0.1, yh[:, 0:kl],
                          OP.mult, OP.max)
                      nc.scalar.activation(hla[64:128, 0:kl], t2[64:128, 0:kl],
                                           AF.Gelu,
                                           bias=pv_t[64:128, 1:2],
                                           scale=pv_t[64:128, 0:1])
                      klo = min(kl, NPC - c0)
                      nc.sync.dma_start(out_d.ap()[1, b, :, c0:c0 + klo],
                                        hla[0:64, 0:klo])
                      nc.sync.dma_start(out_d.ap()[2, b, :, c0:c0 + klo],
                                        hla[64:128, 0:klo])
                      sd = statps.tile([1, 512], f32, tag="sd")
                      nc.tensor.matmul(sd[:, 0:kl], pv_t[:, 2:3],
                                       hla[:, 0:kl],
                                       start=True, stop=True,
                                       skip_group_check=True,
                                       tile_position=(0, 0))
                      sdb = ssbpool.tile([1, 512], f32, tag="sdb")
                      nc.scalar.activation(sdb[:, 0:kl], sd[:, 0:kl], AF.Copy)
                      nc.sync.dma_start(sdt[q:q + 1, 0:kl], sdb[:, 0:kl])

                  # ---- batched 2-way softmax (in-place to save SBUF) ----
                  a0 = smaxpool.tile([GRP, 512], f32, tag="a0")
                  nc.scalar.activation(a0[:], sdt[:], AF.Sigmoid,
                                       bias=bad_t[:])
                  w0 = sdt
                  nc.vector.tensor_scalar_add(w0[:], a0[:], 0.3)
                  w1_ = a0
                  nc.scalar.activation(w1_[:], a0[:], AF.Identity,
                                       bias=c13_t[:], scale=-1.0)

                  # ---- per-pair fusion + residual + output ----
                  for q, (b, c0, kl) in gpairs:
                      hla = hla_t[q]
                      sr = sr_t[q]
                      w0s = stgpool.tile([1, 512], f32, tag="w0s")
                      nc.sync.dma_start(w0s[:, 0:kl], w0[q:q + 1, 0:kl])
                      w1s = stgpool.tile([1, 512], f32, tag="w1s")
                      nc.sync.dma_start(w1s[:, 0:kl], w1_[q:q + 1, 0:kl])
                      wbc = ebcps.tile([128, 512], f32, tag="ebc")
                      nc.tensor.matmul(wbc[:, 0:kl], ehl[0:1, :],
                                       w0s[:, 0:kl], start=True,
                                       stop=False, skip_group_check=True,
                                       tile_position=(0, 0))
                      nc.tensor.matmul(wbc[:, 0:kl], ec1[:],
                                       w1s[:, 0:kl], start=False,
                                       stop=True, skip_group_check=True,
                                       tile_position=(0, 0))
                      f1 = tmppool.tile([128, 512], f32, tag="f1")
                      nc.vector.tensor_mul(f1[:, 0:kl], hla[:, 0:kl],
                                           wbc[:, 0:kl])
                      rrs = stgpool.tile([1, 512], f32, tag="rrs")
                      nc.gpsimd.dma_start(rrs[:, 0:kl],
                                          rstd[4 * q + 2:4 * q + 3, 0:kl])
                      rms = stgpool.tile([1, 512], f32, tag="rms")
                      nc.gpsimd.dma_start(rms[:, 0:kl],
                                          mrstd[4 * q + 2:4 * q + 3, 0:kl])
                      rr = ebcps.tile([64, 512], f32, tag="ebc")
                      nc.tensor.matmul(rr[:, 0:kl], onesr_t[:],
                                       rrs[:, 0:kl],
                                       start=True, stop=True,
                                       skip_group_check=True,
                                       tile_position=(0, 0))
                      rm = ebcps.tile([64, 512], f32, tag="ebc")
                      nc.tensor.matmul(rm[:, 0:kl], onesr_t[:],
                                       rms[:, 0:kl],
                                       start=True, stop=True,
                                       skip_group_check=True,
                                       tile_position=(0, 0))
                      u1 = tmppool.tile([64, 512], f32, tag="u1")
                      nc.vector.tensor_mul(u1[:, 0:kl], sr[0:64, 0:kl],
                                           rr[:, 0:kl])
                      u2 = tmppool.tile([64, 512], f32, tag="u2")
                      nc.vector.tensor_sub(u2[:, 0:kl], u1[:, 0:kl],
                                           rm[:, 0:kl])
                      resa = tmppool.tile([64, 512], f32, tag="resa")
                      nc.scalar.activation(resa[:, 0:kl], u2[:, 0:kl],
                                           AF.Identity,
                                           bias=pv_t[0:64, 4:5],
                                           scale=pv_t[0:64, 3:4])
                      f2 = ebcps.tile([64, 512], f32, tag="ebc")
                      nc.tensor.matmul(f2[:, 0:kl], ident2[:], f1[:, 0:kl],
                                       start=True, stop=True,
                                       skip_group_check=True,
                                       tile_position=(0, 0))
                      f3 = tmppool.tile([64, 512], f32, tag="f3")
                      nc.vector.tensor_add(f3[:, 0:kl], f2[:, 0:kl],
                                           resa[:, 0:kl])
                      klo = min(kl, NPC - c0)
                      nc.sync.dma_start(out_d.ap()[0, b, :, c0:c0 + klo],
                                        f3[:, 0:klo])
    nc.finalize()
    return nc


# ------------------------------------------------------------------- runner
class _SpmdRunner:
    def __init__(self, nc, n_cores=NCORE):
        import jax
        from jax.sharding import Mesh, PartitionSpec
        from jax.experimental.shard_map import shard_map
        from concourse import mybir
        from concourse.bass2jax import (_bass_exec_p, install_neuronx_cc_hook,
                                        partition_id_tensor)
        install_neuronx_cc_hook()
        self.jax = jax
        self.n_cores = n_cores
        partition_name = (nc.partition_id_tensor.name
                          if nc.partition_id_tensor else None)
        in_names, out_names, out_avals = [], [], []
        for alloc in nc.m.functions[0].allocations:
            if not isinstance(alloc, mybir.MemoryLocationSet):
                continue
            name = alloc.memorylocations[0].name
            if alloc.kind == "ExternalInput":
                if name != partition_name:
                    in_names.append(name)
            elif alloc.kind == "ExternalOutput":
                out_names.append(name)
                shape = tuple(alloc.tensor_shape)
                dtype = mybir.dt.np(alloc.dtype)
                out_avals.append(jax.core.ShapedArray(shape, dtype))
        self.in_names, self.out_names = in_names, out_names
        self.out_avals = out_avals
        n_params = len(in_names)
        # The kernel writes every element of every output, so the pre-zeroed
        # output operands of the stock runner are dropped — they would be
        # re-streamed to the terminal on every execute.
        all_in = list(in_names)
        if partition_name is not None:
            all_in.append(partition_name)

        def _body(*args):
            operands = list(args)
            if partition_name is not None:
                operands.append(partition_id_tensor())
            outs = _bass_exec_p.bind(
                *operands, out_avals=tuple(out_avals),
                in_names=tuple(all_in), out_names=tuple(out_names),
                lowering_input_output_aliases=(),
                sim_require_finite=True, sim_require_nnan=True, nc=nc)
            return tuple(outs)

        devices = jax.devices()[:n_cores]
        mesh = Mesh(np.asarray(devices), ("core",))
        in_specs = (PartitionSpec("core"),) * n_params
        out_specs = (PartitionSpec("core"),) * len(out_names)
        self.fn = jax.jit(
            shard_map(_body, mesh=mesh, in_specs=in_specs,
                      out_specs=out_specs, check_rep=False),
            keep_unused=True)

    def prepare(self, in_maps):
        n = self.n_cores
        per_core = [[np.ascontiguousarray(m[name]) for name in self.in_names]
                    for m in in_maps]
        concat_in = [np.concatenate([per_core[c][i] for c in range(n)], axis=0)
                     for i in range(len(self.in_names))]
        return [self.jax.device_put(a) for a in concat_in]

    def run(self, args):
        outs = self.fn(*args)
        self.jax.block_until_ready(outs)
        return outs

    def split_outs(self, outs):
        res = []
        for c in range(self.n_cores):
            d = {}
            for i, name in enumerate(self.out_names):
                d[name] = np.asarray(outs[i]).reshape(
                    self.n_cores, *self.out_avals[i].shape)[c]
            res.append(d)
        return res


# -------------------------------------------------------------------- entry
def _get(inputs):
    gb = _prep_graph(inputs['edge_index'])
    gc = _prep_graph(inputs['causal_edge_index'])
    b_pad = max(128, -(-int(max(gb['counts'].max(), gc['counts'].max()))
                     // 128) * 128)
    stream = NBLK * b_pad
    nseg = stream // 128
    nch = -(-stream // 1024)
    pw, pv, ba_diff = _pack_weights(inputs)
    key = (b_pad, nseg, nch, round(ba_diff, 9), REPEAT,
           NQ_ALLOC, NQ_SPREAD, XTS_SHARED)
    if key not in _cache:
        nc = _build_program(nseg, nch, ba_diff, REPEAT)
        _cache[key] = _SpmdRunner(nc)
    return _cache[key], gb, gc, b_pad, nseg, nch, pw, pv


def make_in_maps(inputs):
    runner, gb, gc, b_pad, nseg, nch, pw, pv = _get(inputs)
    x = np.asarray(inputs['x'], np.float32)
    xflat = x.reshape(96, N)
    idx_b, dli_b = _build_streams(gb, b_pad, nseg, nch)
    idx_c, dli_c = _build_streams(gc, b_pad, nseg, nch)
    h = lambda a: np.ascontiguousarray(a).view(np.float16).ravel()
    in_maps = []
    for c in range(NCORE):
        xsh = np.zeros((96, NPCP), np.float16)
        xsh[:, 0:NPC] = xflat[:, c * NPC:(c + 1) * NPC]
        blob = np.concatenate([
            h(xsh), h(pw), h(pv),
            h(idx_b[c]), h(idx_c[c]),
            h(dli_b[c]), h(dli_c[c]),
        ])[None, :]
        in_maps.append({'blob': blob})
    return runner, in_maps


def kernel(**inputs):
    runner, in_maps = make_in_maps(inputs)
    args = runner.prepare(in_maps)
    outs = runner.run(args)
    res = runner.split_outs(outs)
    fused = np.empty((B, OD, N), np.float32)
    high = np.empty((B, OD, N), np.float32)
    low = np.empty((B, OD, N), np.float32)
    for c in range(NCORE):
        sl = slice(c * NPC, (c + 1) * NPC)
        o = res[c]['out'].astype(np.float32)
        fused[:, :, sl] = o[0]
        high[:, :, sl] = o[1]
        low[:, :, sl] = o[2]
    return fused, high, low

